# revision 1
# baseline (speedup 1.0000x reference)
"""Trainium2 Bass kernel for nn_Conv2d (B=32, 256->256, 56x56, 3x3, pad=1) + bias.

Strategy
--------
Data-parallel over batch: 4 images per NeuronCore x 8 cores; weights/bias
replicated; no collectives.

Per core, the conv is computed as shifted matmuls: the input is zero-padded on
the HOST to 58-wide rows (59 rows x 58 cols per image-channel, flattened to
3422), so output position (h, w) <-> flat index h*58+w, and the 3x3 tap
(kh, kw) contribution is a matmul against the padded input shifted by the
constant offset kh*58+kw.  Each output tile [128 couts x 464 positions]
accumulates 2 (cin chunks) x 9 (taps) = 18 matmuls in one PSUM bank
(3248 = 7*464 padded output positions per image; columns w in {56,57} are
junk and stripped on the host).  Matmuls run as float32r (1 cycle/row on the
PE at N>=256; ~1.4e-4 relative error, measured on HW).  Bias is fused into
the PSUM->SBUF eviction via ScalarE activation(Identity, bias=...).
"""

import numpy as np

import concourse.bacc as bacc
import concourse.tile as tile
import concourse.mybir as mybir
from concourse.bass_utils import run_bass_kernel_spmd

F32 = mybir.dt.float32
F32R = mybir.dt.float32r

B, CIN, COUT, H, W, K = 32, 256, 256, 56, 56, 3
NCORES = 8
BPC = B // NCORES          # images per core
WP = W + 2                 # padded row width (58)
HP = H + 3                 # padded rows (59): 1 top, 2 bottom (tail tap reads)
XF = HP * WP               # padded flat length per image-channel (3422)
OF = H * WP                # padded output flat length (3248)
NT = 7                     # output tiles per (img, cout-chunk)
NFREE = OF // NT           # 464 positions per matmul (>=256 keeps f32r fast)

_CACHE = {}


def _build():
    if "nc" in _CACHE:
        return _CACHE["nc"]
    nc = bacc.Bacc("TRN2", target_bir_lowering=False, debug=False,
                   num_swdge_queues=4)
    x_d = nc.dram_tensor("x", [BPC, CIN, XF], F32R, kind="ExternalInput").ap()
    w_d = nc.dram_tensor("w", [K * K, CIN, COUT], F32R, kind="ExternalInput").ap()
    b_d = nc.dram_tensor("b", [COUT], F32, kind="ExternalInput").ap()
    o_d = nc.dram_tensor("o", [BPC, COUT, OF], F32, kind="ExternalOutput").ap()

    XLOAD = 3366  # matmuls never read past 3365

    with tile.TileContext(nc) as tc:
        with (
            tc.tile_pool(name="wp", bufs=1) as wp,
            tc.tile_pool(name="xp", bufs=6) as xp,
            tc.tile_pool(name="op", bufs=2) as op,
            tc.tile_pool(name="pp", bufs=8, space="PSUM") as pp,
        ):
            # DMA trigger instructions cost ~0.6us EACH on the issuing
            # engine, so issue in parallel from both HWDGE engines:
            # sync carries ci=0 traffic, scalar carries ci=1.
            eng = [nc.sync, nc.scalar]

            bias_t = wp.tile([128, 2], F32)
            # weights [cin-in-chunk, cin_chunk, tap, cout] in per-(cc,ci,tap)
            # 64KB DMAs: the cc=0 half (1.18MB) is all the first compute wave
            # needs; cc=1 arrives during it.
            w_t = wp.tile([128, 2, K * K, COUT], F32R)

            def w_dma(e, ci, t, cc):
                e.dma_start(
                    out=w_t[:, ci, t, cc * 128:(cc + 1) * 128],
                    in_=w_d[t, ci * 128:(ci + 1) * 128, cc * 128:(cc + 1) * 128],
                )

            def x_dma(e, xs, img, ci, lo, hi):
                e.dma_start(
                    out=xs[ci][:, lo:hi],
                    in_=x_d[img, ci * 128:(ci + 1) * 128, lo:hi],
                )

            def alloc_x():
                xs = []
                for ci in range(2):
                    x_t = xp.tile([128, XF], F32R, tag="x")
                    xs.append(x_t)
                return xs

            # slice boundaries: nt-pair p depends only on x up to
            # 582+464*(2p+1), so early pairs unblock as slices land
            xsl = [0, 291, 582, 1046, 1510, 1974, 2438, 2902, XLOAD]

            def load_img(img):
                xs = alloc_x()
                for s in range(len(xsl) - 1):
                    for ci in range(2):
                        x_dma(eng[ci], xs, img, ci, xsl[s], xsl[s + 1])
                return xs

            def load_img0():
                # Hand-scheduled startup: DMA triggers cost ~0.65us each on
                # the issuing engine; sync (ci=0) and scalar (ci=1) carry
                # first-wave weights + x interleaved by consumption time
                xs = alloc_x()
                for ci in range(2):
                    e = eng[ci]
                    w_dma(e, ci, 0, 0)
                    x_dma(e, xs, 0, ci, xsl[0], xsl[1])
                    x_dma(e, xs, 0, ci, xsl[1], xsl[2])
                    x_dma(e, xs, 0, ci, xsl[2], xsl[3])
                    w_dma(e, ci, 1, 0)
                    w_dma(e, ci, 2, 0)
                    x_dma(e, xs, 0, ci, xsl[3], xsl[4])
                    w_dma(e, ci, 3, 0)
                    w_dma(e, ci, 4, 0)
                    x_dma(e, xs, 0, ci, xsl[4], xsl[5])
                    w_dma(e, ci, 5, 0)
                    w_dma(e, ci, 6, 0)
                    x_dma(e, xs, 0, ci, xsl[5], xsl[6])
                    w_dma(e, ci, 7, 0)
                    w_dma(e, ci, 8, 0)
                    e.dma_start(out=bias_t[:, ci:ci + 1],
                                in_=b_d[ci * 128:(ci + 1) * 128])
                    x_dma(e, xs, 0, ci, xsl[6], xsl[7])
                    x_dma(e, xs, 0, ci, xsl[7], xsl[8])
                return xs

            def do_group(xs, cc, o_t, img, nts, fine_stores=False):
                """One PSUM accumulation wave over nt tiles `nts` (1 or 2),
                sharing each weight tile across the wave to halve LDWEIGHTS
                pressure on the PE."""
                pss = []
                for nt in nts:
                    ps = pp.tile([128, NFREE], F32, tag="ps")
                    pss.append(ps)
                for mi, (ci, t) in enumerate(
                    [(ci, t) for ci in range(2) for t in range(K * K)]
                ):
                    kh, kw = divmod(t, K)
                    for ps, nt in zip(pss, nts):
                        off = nt * NFREE + kh * WP + kw
                        nc.tensor.matmul(
                            ps,
                            w_t[:, ci, t, cc * 128:(cc + 1) * 128],
                            xs[ci][:, off:off + NFREE],
                            start=(mi == 0),
                            stop=(mi == 17),
                        )
                for ps, nt in zip(pss, nts):
                    # bias-add + PSUM eviction on the otherwise-idle DVE
                    nc.vector.tensor_scalar_add(
                        o_t[:, nt * NFREE:(nt + 1) * NFREE],
                        ps,
                        bias_t[:, cc:cc + 1],
                    )
                    # store each nt slice as soon as its bias-add finishes,
                    # halves split across the issue engines (quarters for
                    # the final group so the drain tail stays short)
                    nsp = 4 if fine_stores else 2
                    q = NFREE // nsp
                    for s in range(nsp):
                        h0 = nt * NFREE + s * q
                        eng[s % 2].dma_start(
                            out=o_d[img, cc * 128:(cc + 1) * 128, h0:h0 + q],
                            in_=o_t[:, h0:h0 + q],
                        )

            for img in range(BPC):
                if img == 0:
                    xs = load_img0()
                    # cc=1 weights via SWDGE, needed ~27us in
                    for ci in range(2):
                        for t in range(K * K):
                            w_dma(nc.gpsimd, ci, t, 1)
                else:
                    xs = load_img(img)
                for cc in range(2):
                    o_t = op.tile([128, OF], F32, tag="o")
                    last = img == BPC - 1 and cc == 1
                    for nts in ([0, 1], [2, 3], [4, 5], [6]):
                        do_group(xs, cc, o_t, img, nts,
                                 fine_stores=last and nts == [6])
    nc.compile()
    _CACHE["nc"] = nc
    return nc


def make_in_maps(inp, kernel, bias):
    xpad = np.zeros((B, CIN, HP, WP), np.float32)
    xpad[:, :, 1:1 + H, 1:1 + W] = inp
    xflat = xpad.reshape(B, CIN, XF)
    # [cout, cin, kh, kw] -> [tap(kh*3+kw), cin, cout]
    w_dev = np.ascontiguousarray(
        np.asarray(kernel, np.float32).transpose(2, 3, 1, 0).reshape(K * K, CIN, COUT)
    )
    b_dev = np.ascontiguousarray(np.asarray(bias, np.float32))
    return [
        {"x": np.ascontiguousarray(xflat[c * BPC:(c + 1) * BPC]),
         "w": w_dev, "b": b_dev}
        for c in range(NCORES)
    ]


def assemble(results):
    o = np.concatenate([results[c]["o"] for c in range(NCORES)], axis=0)
    return np.ascontiguousarray(
        o.reshape(B, COUT, H, WP)[:, :, :, :W].astype(np.float32)
    )


def kernel(inp, kernel, bias):
    nc = _build()
    in_maps = make_in_maps(inp, kernel, bias)
    r = run_bass_kernel_spmd(nc, in_maps, core_ids=list(range(NCORES)))
    return assemble(r.results)



# revision 4
# speedup vs baseline: 1.2564x; 1.2564x over previous
"""Trainium2 Bass kernel for nn_Conv2d (B=32, 256->256, 56x56, 3x3, pad=1) + bias.

Strategy
--------
Data-parallel over batch: 4 images per NeuronCore x 8 cores; weights/bias
replicated; no collectives.

Per core, the conv is computed as shifted matmuls: the input is zero-padded on
the HOST to 58-wide rows (59 rows x 58 cols per image-channel, flattened to
3422), so output position (h, w) <-> flat index h*58+w, and the 3x3 tap
(kh, kw) contribution is a matmul against the padded input shifted by the
constant offset kh*58+kw.  Operands are bf16 (rel err ~3e-3 vs the 2e-2
gate), which unlocks a standalone LDWEIGHTS shared by a *group* of matmuls:
each of the 18 (cin-chunk, tap) weight tiles is loaded once per group of 4
(or 3) PSUM banks, so the PE pays the weight-swap bubble once per ~4 matmuls
instead of per matmul (f32r matmuls must self-load).  Groups of 4+3 banks
ping-pong across the 8 PSUM banks so evictions (bias-add on DVE, bf16 out)
and stores overlap the next group's matmuls.  Junk columns w in {56,57} of
each 58-wide output row are stripped on the host.
"""

import numpy as np
import ml_dtypes

import concourse.bacc as bacc
import concourse.tile as tile
import concourse.mybir as mybir
from concourse.bass_utils import run_bass_kernel_spmd

F32 = mybir.dt.float32
BF16 = mybir.dt.bfloat16
BF = ml_dtypes.bfloat16

B, CIN, COUT, H, W, K = 32, 256, 256, 56, 56, 3
NCORES = 8
BPC = B // NCORES          # images per core
WP = W + 2                 # padded row width (58)
HP = H + 3                 # padded rows (59): 1 top, 2 bottom (tail tap reads)
XF = HP * WP               # padded flat length per image-channel (3422)
OF = H * WP                # padded output flat length (3248)
NT = 7                     # output tiles per (img, cout-chunk)
NFREE = OF // NT           # 464 positions per matmul
XLOAD = 3366               # matmuls never read past 3365
NW = K * K * 128           # weight free length per (ci, cc): 9 taps x 128 couts

_CACHE = {}


def _build():
    if "nc" in _CACHE:
        return _CACHE["nc"]
    nc = bacc.Bacc("TRN2", target_bir_lowering=False, debug=False,
                   num_swdge_queues=4)
    x_d = nc.dram_tensor("x", [BPC, CIN, XF], BF16, kind="ExternalInput").ap()
    w_d = nc.dram_tensor("w", [2, 128, 2, NW], BF16, kind="ExternalInput").ap()
    b_d = nc.dram_tensor("b", [COUT], F32, kind="ExternalInput").ap()
    o_d = nc.dram_tensor("o", [BPC, COUT, OF], BF16, kind="ExternalOutput").ap()

    with tile.TileContext(nc) as tc:
        with (
            tc.tile_pool(name="wp", bufs=1) as wp,
            tc.tile_pool(name="xp", bufs=6) as xp,
            tc.tile_pool(name="op", bufs=2) as op,
            tc.tile_pool(name="pp", bufs=8, space="PSUM") as pp,
        ):
            # DMA trigger instructions cost ~0.7us EACH on the issuing
            # engine, so issue in parallel from both HWDGE engines:
            # sync carries ci=0 traffic, scalar carries ci=1.
            eng = [nc.sync, nc.scalar]

            bias_t = wp.tile([128, 2], F32)
            # weights [cin-in-chunk, ci, cc, tap*128+cout]: one contiguous
            # 295KB DMA per (ci, cc) chunk.
            w_t = wp.tile([128, 2, 2, NW], BF16)

            def w_dma(e, ci, cc):
                e.dma_start(out=w_t[:, ci, cc, :], in_=w_d[ci, :, cc, :])

            def x_dma(e, xs, img, ci, lo, hi):
                e.dma_start(
                    out=xs[ci][:, lo:hi],
                    in_=x_d[img, ci * 128:(ci + 1) * 128, lo:hi],
                )

            # img0: fine front slices so the PE starts ASAP; steady images:
            # 4 coarse slices (they prefetch a whole image ahead anyway)
            xsl0 = [0, 291, 582, 1046, 1510, 1974, 2438, 2902, XLOAD]
            xsl = [0, 846, 1692, 2538, XLOAD]

            def load_img(img, first=False):
                xs = [xp.tile([128, XF], BF16, tag="x", name=f"x_{img}_{ci}")
                      for ci in range(2)]
                for ci in range(2):
                    e = eng[ci]
                    if first:
                        w_dma(e, ci, 0)
                        for s in range(3):
                            x_dma(e, xs, img, ci, xsl0[s], xsl0[s + 1])
                        w_dma(e, ci, 1)
                        e.dma_start(out=bias_t[:, ci:ci + 1],
                                    in_=b_d[ci * 128:(ci + 1) * 128])
                        for s in range(3, len(xsl0) - 1):
                            x_dma(e, xs, img, ci, xsl0[s], xsl0[s + 1])
                    else:
                        for s in range(len(xsl) - 1):
                            x_dma(e, xs, img, ci, xsl[s], xsl[s + 1])
                return xs

            def do_pass(xs, cc, o_t, img, nts, fine=False):
                """One PSUM accumulation wave over banks `nts`: each of the
                18 (ci, tap) weight tiles is LDWEIGHTS'd once and streamed
                through len(nts) matmuls (ldweights=False on all of them)."""
                pss = [pp.tile([128, NFREE], F32, tag="ps",
                               name=f"ps_{img}_{cc}_{nt}") for nt in nts]
                for mi, (ci, t) in enumerate(
                    [(ci, t) for ci in range(2) for t in range(K * K)]
                ):
                    kh, kw = divmod(t, K)
                    wsl = w_t[:, ci, cc, t * 128:(t + 1) * 128]
                    nc.tensor.ldweights(wsl)
                    for ps, nt in zip(pss, nts):
                        off = nt * NFREE + kh * WP + kw
                        mm = nc.tensor.matmul(
                            ps, wsl, xs[ci][:, off:off + NFREE],
                            start=(mi == 0), stop=(mi == 17),
                        )
                        mm.ins.ldweights = False
                # bias-add + PSUM eviction on the otherwise-idle DVE,
                # bf16 out halves store traffic
                for j, (ps, nt) in enumerate(zip(pss, nts)):
                    nc.vector.tensor_scalar_add(
                        o_t[:, nt * NFREE:(nt + 1) * NFREE],
                        ps,
                        bias_t[:, cc:cc + 1],
                    )
                    if fine:
                        eng[j % 2].dma_start(
                            out=o_d[img, cc * 128:(cc + 1) * 128,
                                    nt * NFREE:(nt + 1) * NFREE],
                            in_=o_t[:, nt * NFREE:(nt + 1) * NFREE],
                        )
                if not fine:
                    lo, hi = nts[0] * NFREE, (nts[-1] + 1) * NFREE
                    eng[nts[0] % 2].dma_start(
                        out=o_d[img, cc * 128:(cc + 1) * 128, lo:hi],
                        in_=o_t[:, lo:hi],
                    )

            for img in range(BPC):
                xs = load_img(img, first=(img == 0))
                for cc in range(2):
                    o_t = op.tile([128, OF], BF16, tag="o")
                    last = img == BPC - 1 and cc == 1
                    do_pass(xs, cc, o_t, img, [0, 1, 2, 3])
                    do_pass(xs, cc, o_t, img, [4, 5, 6], fine=last)
    nc.compile()
    _CACHE["nc"] = nc
    return nc


def make_in_maps(inp, kernel, bias):
    xpad = np.zeros((B, CIN, HP, WP), np.float32)
    xpad[:, :, 1:1 + H, 1:1 + W] = inp
    xflat = xpad.reshape(B, CIN, XF).astype(BF)
    # [cout, cin, kh, kw] -> [ci, cin_in, cc, tap*128+cout_in]
    kk = np.asarray(kernel, np.float32).reshape(2, 128, 2, 128, K, K)
    w_dev = np.ascontiguousarray(
        kk.transpose(2, 3, 0, 4, 5, 1).reshape(2, 128, 2, NW)
    ).astype(BF)
    b_dev = np.ascontiguousarray(np.asarray(bias, np.float32))
    return [
        {"x": np.ascontiguousarray(xflat[c * BPC:(c + 1) * BPC]),
         "w": w_dev, "b": b_dev}
        for c in range(NCORES)
    ]


def assemble(results):
    o = np.concatenate([np.asarray(results[c]["o"]) for c in range(NCORES)],
                       axis=0)
    return np.ascontiguousarray(
        o.reshape(B, COUT, H, WP)[:, :, :, :W].astype(np.float32)
    )


def kernel(inp, kernel, bias):
    nc = _build()
    in_maps = make_in_maps(inp, kernel, bias)
    r = run_bass_kernel_spmd(nc, in_maps, core_ids=list(range(NCORES)))
    return assemble(r.results)


# revision 8
# speedup vs baseline: 1.2815x; 1.0200x over previous
"""Trainium2 Bass kernel for nn_Conv2d (B=32, 256->256, 56x56, 3x3, pad=1) + bias.

Strategy
--------
Data-parallel over batch: 4 images per NeuronCore x 8 cores; weights/bias
replicated; no collectives.

Per core, the conv is computed as shifted matmuls: the input is zero-padded on
the HOST to 58-wide rows (59 rows x 58 cols per image-channel, flattened to
3422), so output position (h, w) <-> flat index h*58+w, and the 3x3 tap
(kh, kw) contribution is a matmul against the padded input shifted by the
constant offset kh*58+kw.  Operands are bf16 (rel err ~3e-3 vs the 2e-2
gate), which unlocks a standalone LDWEIGHTS shared by a *group* of matmuls:
each of the 18 (cin-chunk, tap) weight tiles is loaded once per group of 4
(or 3) PSUM banks, so the PE pays the weight-swap bubble once per ~4 matmuls
instead of per matmul (f32r matmuls must self-load).  Groups of 4+3 banks
ping-pong across the 8 PSUM banks so evictions (bias-add on DVE, bf16 out)
and stores overlap the next group's matmuls.  Junk columns w in {56,57} of
each 58-wide output row are stripped on the host.
"""

import numpy as np
import ml_dtypes

import concourse.bacc as bacc
import concourse.tile as tile
import concourse.mybir as mybir
from concourse.bass_utils import run_bass_kernel_spmd

F32 = mybir.dt.float32
BF16 = mybir.dt.bfloat16
BF = ml_dtypes.bfloat16

B, CIN, COUT, H, W, K = 32, 256, 256, 56, 56, 3
NCORES = 8
BPC = B // NCORES          # images per core
WP = W + 2                 # padded row width (58)
HP = H + 3                 # padded rows (59): 1 top, 2 bottom (tail tap reads)
XF = HP * WP               # padded flat length per image-channel (3422)
OF = H * WP                # padded output flat length (3248)
NT = 7                     # output tiles per (img, cout-chunk)
NFREE = OF // NT           # 464 positions per matmul
XLOAD = 3366               # matmuls never read past 3365
NW = K * K * 128           # weight free length per (ci, cc): 9 taps x 128 couts

_CACHE = {}


def _build():
    if "nc" in _CACHE:
        return _CACHE["nc"]
    nc = bacc.Bacc("TRN2", target_bir_lowering=False, debug=False,
                   num_swdge_queues=4)
    x_d = nc.dram_tensor("x", [BPC, CIN, XF], BF16, kind="ExternalInput").ap()
    w_d = nc.dram_tensor("w", [2, 128, 2, NW], BF16, kind="ExternalInput").ap()
    b_d = nc.dram_tensor("b", [COUT], F32, kind="ExternalInput").ap()
    o_d = nc.dram_tensor("o", [BPC, COUT, OF], BF16, kind="ExternalOutput").ap()

    with tile.TileContext(nc) as tc:
        with (
            tc.tile_pool(name="wp", bufs=1) as wp,
            tc.tile_pool(name="xp", bufs=6) as xp,
            tc.tile_pool(name="op", bufs=2) as op,
            tc.tile_pool(name="pp", bufs=8, space="PSUM") as pp,
        ):
            # DMA trigger instructions cost ~0.7us EACH on the issuing
            # engine, so issue in parallel from both HWDGE engines:
            # sync carries ci=0 traffic, scalar carries ci=1.
            eng = [nc.sync, nc.scalar]

            bias_t = wp.tile([128, 2], F32)
            # weights [cin-in-chunk, ci, cc, tap*128+cout]: one contiguous
            # 295KB DMA per (ci, cc) chunk.
            w_t = wp.tile([128, 2, 2, NW], BF16)

            def w_dma(e, ci, cc, lo=0, hi=K * K):
                e.dma_start(out=w_t[:, ci, cc, lo * 128:hi * 128],
                            in_=w_d[ci, :, cc, lo * 128:hi * 128])

            def x_dma(e, xs, img, ci, lo, hi):
                e.dma_start(
                    out=xs[ci][:, lo:hi],
                    in_=x_d[img, ci * 128:(ci + 1) * 128, lo:hi],
                )

            # img0: fine front slices so the PE starts ASAP; steady images:
            # 4 coarse slices (they prefetch a whole image ahead anyway)
            xsl0 = [0, 291, 582, 1046, 1510, 1974, 2438, 2902, XLOAD]
            xsl = [0, 846, 1692, 2538, XLOAD]

            def load_img(img, first=False):
                xs = [xp.tile([128, XF], BF16, tag="x", name=f"x_{img}_{ci}")
                      for ci in range(2)]
                for ci in range(2):
                    e = eng[ci]
                    if first:
                        # taps 0-2 first (98KB) so the PE starts ~2us sooner
                        w_dma(e, ci, 0, 0, 3)
                        x_dma(e, xs, img, ci, xsl0[0], xsl0[1])
                        w_dma(e, ci, 0, 3, K * K)
                        for s in range(1, 3):
                            x_dma(e, xs, img, ci, xsl0[s], xsl0[s + 1])
                        w_dma(e, ci, 1)
                        e.dma_start(out=bias_t[:, ci:ci + 1],
                                    in_=b_d[ci * 128:(ci + 1) * 128])
                        for s in range(3, len(xsl0) - 1):
                            x_dma(e, xs, img, ci, xsl0[s], xsl0[s + 1])
                    else:
                        for s in range(len(xsl) - 1):
                            x_dma(e, xs, img, ci, xsl[s], xsl[s + 1])
                return xs

            def do_pass(xs, cc, o_t, img, nts, fine=False):
                """One PSUM accumulation wave over banks `nts`: each of the
                18 (ci, tap) weight tiles is LDWEIGHTS'd once and streamed
                through len(nts) matmuls (ldweights=False on all of them)."""
                pss = [pp.tile([128, NFREE], F32, tag="ps",
                               name=f"ps_{img}_{cc}_{nt}") for nt in nts]
                for mi, (ci, t) in enumerate(
                    [(ci, t) for ci in range(2) for t in range(K * K)]
                ):
                    kh, kw = divmod(t, K)
                    wsl = w_t[:, ci, cc, t * 128:(t + 1) * 128]
                    for ps, nt in zip(pss, nts):
                        off = nt * NFREE + kh * WP + kw
                        nc.tensor.matmul(
                            ps, wsl, xs[ci][:, off:off + NFREE],
                            start=(mi == 0), stop=(mi == 17),
                        )
                # bias-add + PSUM eviction on the otherwise-idle DVE,
                # bf16 out halves store traffic
                for j, (ps, nt) in enumerate(zip(pss, nts)):
                    nc.vector.tensor_scalar_add(
                        o_t[:, nt * NFREE:(nt + 1) * NFREE],
                        ps,
                        bias_t[:, cc:cc + 1],
                    )
                    if fine:
                        eng[j % 2].dma_start(
                            out=o_d[img, cc * 128:(cc + 1) * 128,
                                    nt * NFREE:(nt + 1) * NFREE],
                            in_=o_t[:, nt * NFREE:(nt + 1) * NFREE],
                        )
                if not fine:
                    lo, hi = nts[0] * NFREE, (nts[-1] + 1) * NFREE
                    eng[nts[0] % 2].dma_start(
                        out=o_d[img, cc * 128:(cc + 1) * 128, lo:hi],
                        in_=o_t[:, lo:hi],
                    )

            for img in range(BPC):
                xs = load_img(img, first=(img == 0))
                for cc in range(2):
                    o_t = op.tile([128, OF], BF16, tag="o")
                    last = img == BPC - 1 and cc == 1
                    if last:
                        # taper the final passes so the drain tail is short
                        do_pass(xs, cc, o_t, img, [0, 1, 2, 3])
                        do_pass(xs, cc, o_t, img, [4, 5], fine=True)
                        do_pass(xs, cc, o_t, img, [6], fine=True)
                    else:
                        do_pass(xs, cc, o_t, img, [0, 1, 2, 3])
                        do_pass(xs, cc, o_t, img, [4, 5, 6])
    nc.compile()
    _CACHE["nc"] = nc
    return nc


def make_in_maps(inp, kernel, bias):
    xpad = np.zeros((B, CIN, HP, WP), np.float32)
    xpad[:, :, 1:1 + H, 1:1 + W] = inp
    xflat = xpad.reshape(B, CIN, XF).astype(BF)
    # [cout, cin, kh, kw] -> [ci, cin_in, cc, tap*128+cout_in]
    kk = np.asarray(kernel, np.float32).reshape(2, 128, 2, 128, K, K)
    w_dev = np.ascontiguousarray(
        kk.transpose(2, 3, 0, 4, 5, 1).reshape(2, 128, 2, NW)
    ).astype(BF)
    b_dev = np.ascontiguousarray(np.asarray(bias, np.float32))
    return [
        {"x": np.ascontiguousarray(xflat[c * BPC:(c + 1) * BPC]),
         "w": w_dev, "b": b_dev}
        for c in range(NCORES)
    ]


def assemble(results):
    o = np.concatenate([np.asarray(results[c]["o"]) for c in range(NCORES)],
                       axis=0)
    return np.ascontiguousarray(
        o.reshape(B, COUT, H, WP)[:, :, :, :W].astype(np.float32)
    )


def kernel(inp, kernel, bias):
    nc = _build()
    in_maps = make_in_maps(inp, kernel, bias)
    r = run_bass_kernel_spmd(nc, in_maps, core_ids=list(range(NCORES)))
    return assemble(r.results)


# revision 12
# speedup vs baseline: 1.2970x; 1.0121x over previous
"""Trainium2 Bass kernel for nn_Conv2d (B=32, 256->256, 56x56, 3x3, pad=1) + bias.

Strategy
--------
Data-parallel over batch: 4 images per NeuronCore x 8 cores; weights/bias
replicated; no collectives.

Per core, the conv is computed as shifted matmuls: the input is zero-padded on
the HOST to 58-wide rows (59 rows x 58 cols per image-channel, flattened to
3422), so output position (h, w) <-> flat index h*58+w, and the 3x3 tap
(kh, kw) contribution is a matmul against the padded input shifted by the
constant offset kh*58+kw.  Operands are bf16 (rel err ~3e-3 vs the 2e-2
gate), which unlocks a standalone LDWEIGHTS shared by a *group* of matmuls:
each of the 18 (cin-chunk, tap) weight tiles is loaded once per group of 4
(or 3) PSUM banks, so the PE pays the weight-swap bubble once per ~4 matmuls
instead of per matmul (f32r matmuls must self-load).  Groups of 4+3 banks
ping-pong across the 8 PSUM banks so evictions (bias-add on DVE, bf16 out)
and stores overlap the next group's matmuls.  Junk columns w in {56,57} of
each 58-wide output row are stripped on the host.
"""

import numpy as np
import ml_dtypes

import concourse.bacc as bacc
import concourse.tile as tile
import concourse.mybir as mybir
from concourse.bass_utils import run_bass_kernel_spmd

F32 = mybir.dt.float32
BF16 = mybir.dt.bfloat16
BF = ml_dtypes.bfloat16

B, CIN, COUT, H, W, K = 32, 256, 256, 56, 56, 3
NCORES = 8
BPC = B // NCORES          # images per core
WP = W + 2                 # padded row width (58)
HP = H + 3                 # padded rows (59): 1 top, 2 bottom (tail tap reads)
XF = HP * WP               # padded flat length per image-channel (3422)
OF = H * WP                # padded output flat length (3248)
NT = 7                     # output tiles per (img, cout-chunk)
NFREE = OF // NT           # 464 positions per matmul
XLOAD = 3366               # matmuls never read past 3365
NW = K * K * 128           # weight free length per (ci, cc): 9 taps x 128 couts

_CACHE = {}


def _build():
    if "nc" in _CACHE:
        return _CACHE["nc"]
    nc = bacc.Bacc("TRN2", target_bir_lowering=False, debug=False,
                   num_swdge_queues=1)
    x_d = nc.dram_tensor("x", [BPC, CIN, XF], BF16, kind="ExternalInput").ap()
    w_d = nc.dram_tensor("w", [2, 128, 2, NW], BF16, kind="ExternalInput").ap()
    b_d = nc.dram_tensor("b", [COUT], F32, kind="ExternalInput").ap()
    o_d = nc.dram_tensor("o", [BPC, COUT, OF], BF16, kind="ExternalOutput").ap()

    with tile.TileContext(nc) as tc:
        with (
            tc.tile_pool(name="wp", bufs=1) as wp,
            tc.tile_pool(name="xp", bufs=6) as xp,
            tc.tile_pool(name="op", bufs=2) as op,
            tc.tile_pool(name="pp", bufs=8, space="PSUM") as pp,
        ):
            # DMA trigger instructions cost ~0.7us EACH on the issuing
            # engine, so issue in parallel from both HWDGE engines:
            # sync carries ci=0 traffic, scalar carries ci=1.
            eng = [nc.sync, nc.scalar]

            bias_t = wp.tile([128, 2], F32)
            # weights [cin-in-chunk, ci, cc, tap*128+cout]: one contiguous
            # 295KB DMA per (ci, cc) chunk.
            w_t = wp.tile([128, 2, 2, NW], BF16)

            def w_dma(e, ci, cc, lo=0, hi=K * K):
                e.dma_start(out=w_t[:, ci, cc, lo * 128:hi * 128],
                            in_=w_d[ci, :, cc, lo * 128:hi * 128])

            def x_dma(e, xs, img, ci, lo, hi):
                e.dma_start(
                    out=xs[ci][:, lo:hi],
                    in_=x_d[img, ci * 128:(ci + 1) * 128, lo:hi],
                )

            # img0: fine front slices so the PE starts ASAP; steady images:
            # 4 coarse slices (they prefetch a whole image ahead anyway)
            xsl0 = [0, 640, 1046, 1974, 2902, XLOAD]
            xsl = [0, 846, 1692, 2538, XLOAD]

            def load_img(img, first=False):
                xs = [xp.tile([128, XF], BF16, tag="x", name=f"x_{img}_{ci}")
                      for ci in range(2)]
                for ci in range(2):
                    e = eng[ci]
                    if first:
                        # first MM needs x[0:640] + w taps 0-2 only: land
                        # those first, interleave the rest by need-by time
                        x_dma(e, xs, img, ci, xsl0[0], xsl0[1])
                        w_dma(e, ci, 0, 0, 3)
                        x_dma(e, xs, img, ci, xsl0[1], xsl0[2])
                        w_dma(e, ci, 0, 3, K * K)
                        x_dma(e, xs, img, ci, xsl0[2], xsl0[3])
                        w_dma(e, ci, 1)
                        e.dma_start(out=bias_t[:, ci:ci + 1],
                                    in_=b_d[ci * 128:(ci + 1) * 128])
                        for s in range(3, len(xsl0) - 1):
                            x_dma(e, xs, img, ci, xsl0[s], xsl0[s + 1])
                    else:
                        for s in range(len(xsl) - 1):
                            x_dma(e, xs, img, ci, xsl[s], xsl[s + 1])
                return xs

            def do_pass(xs, cc, o_t, img, nts, fine=False):
                """One PSUM accumulation wave over banks `nts`: each of the
                18 (ci, tap) weight tiles is LDWEIGHTS'd once and streamed
                through len(nts) matmuls (ldweights=False on all of them)."""
                pss = [pp.tile([128, NFREE], F32, tag="ps",
                               name=f"ps_{img}_{cc}_{nt}") for nt in nts]
                for mi, (ci, t) in enumerate(
                    [(ci, t) for ci in range(2) for t in range(K * K)]
                ):
                    kh, kw = divmod(t, K)
                    wsl = w_t[:, ci, cc, t * 128:(t + 1) * 128]
                    for ps, nt in zip(pss, nts):
                        off = nt * NFREE + kh * WP + kw
                        nc.tensor.matmul(
                            ps, wsl, xs[ci][:, off:off + NFREE],
                            start=(mi == 0), stop=(mi == 17),
                        )
                # bias-add + PSUM eviction on the otherwise-idle DVE,
                # bf16 out halves store traffic
                for j, (ps, nt) in enumerate(zip(pss, nts)):
                    nc.vector.tensor_scalar_add(
                        o_t[:, nt * NFREE:(nt + 1) * NFREE],
                        ps,
                        bias_t[:, cc:cc + 1],
                    )
                    if fine:
                        eng[j % 2].dma_start(
                            out=o_d[img, cc * 128:(cc + 1) * 128,
                                    nt * NFREE:(nt + 1) * NFREE],
                            in_=o_t[:, nt * NFREE:(nt + 1) * NFREE],
                        )
                if not fine:
                    lo, hi = nts[0] * NFREE, (nts[-1] + 1) * NFREE
                    eng[nts[0] % 2].dma_start(
                        out=o_d[img, cc * 128:(cc + 1) * 128, lo:hi],
                        in_=o_t[:, lo:hi],
                    )

            # warm the PE clock (p-state ramps over ~3us of activity) with
            # dummy matmuls on a memset tile while the first DMAs land
            wu = wp.tile([128, NFREE], BF16)
            nc.gpsimd.memset(wu, 0)
            ps_warm = pp.tile([128, NFREE], F32, tag="ps", name="ps_warm")
            for _ in range(12):
                nc.tensor.matmul(ps_warm, wu[:, 0:128], wu,
                                 start=True, stop=True)

            for img in range(BPC):
                xs = load_img(img, first=(img == 0))
                for cc in range(2):
                    o_t = op.tile([128, OF], BF16, tag="o")
                    last = img == BPC - 1 and cc == 1
                    if last:
                        # taper the final passes so the drain tail is short
                        do_pass(xs, cc, o_t, img, [0, 1, 2, 3])
                        do_pass(xs, cc, o_t, img, [4, 5], fine=True)
                        do_pass(xs, cc, o_t, img, [6], fine=True)
                    else:
                        do_pass(xs, cc, o_t, img, [0, 1, 2, 3])
                        do_pass(xs, cc, o_t, img, [4, 5, 6])
    nc.compile()
    _CACHE["nc"] = nc
    return nc


def make_in_maps(inp, kernel, bias):
    xpad = np.zeros((B, CIN, HP, WP), np.float32)
    xpad[:, :, 1:1 + H, 1:1 + W] = inp
    xflat = xpad.reshape(B, CIN, XF).astype(BF)
    # [cout, cin, kh, kw] -> [ci, cin_in, cc, tap*128+cout_in]
    kk = np.asarray(kernel, np.float32).reshape(2, 128, 2, 128, K, K)
    w_dev = np.ascontiguousarray(
        kk.transpose(2, 3, 0, 4, 5, 1).reshape(2, 128, 2, NW)
    ).astype(BF)
    b_dev = np.ascontiguousarray(np.asarray(bias, np.float32))
    return [
        {"x": np.ascontiguousarray(xflat[c * BPC:(c + 1) * BPC]),
         "w": w_dev, "b": b_dev}
        for c in range(NCORES)
    ]


def assemble(results):
    o = np.concatenate([np.asarray(results[c]["o"]) for c in range(NCORES)],
                       axis=0)
    return np.ascontiguousarray(
        o.reshape(B, COUT, H, WP)[:, :, :, :W].astype(np.float32)
    )


def kernel(inp, kernel, bias):
    nc = _build()
    in_maps = make_in_maps(inp, kernel, bias)
    r = run_bass_kernel_spmd(nc, in_maps, core_ids=list(range(NCORES)))
    return assemble(r.results)


# revision 15
# speedup vs baseline: 1.3072x; 1.0078x over previous
"""Trainium2 Bass kernel for nn_Conv2d (B=32, 256->256, 56x56, 3x3, pad=1) + bias.

Strategy
--------
Data-parallel over batch: 4 images per NeuronCore x 8 cores; weights/bias
replicated; no collectives.

Per core, the conv is computed as shifted matmuls: the input is zero-padded on
the HOST to 58-wide rows (59 rows x 58 cols per image-channel, flattened to
3422), so output position (h, w) <-> flat index h*58+w, and the 3x3 tap
(kh, kw) contribution is a matmul against the padded input shifted by the
constant offset kh*58+kw.  Operands are bf16 (rel err ~3e-3 vs the 2e-2
gate), which unlocks a standalone LDWEIGHTS shared by a *group* of matmuls:
each of the 18 (cin-chunk, tap) weight tiles is loaded once per group of 4
(or 3) PSUM banks, so the PE pays the weight-swap bubble once per ~4 matmuls
instead of per matmul (f32r matmuls must self-load).  Groups of 4+3 banks
ping-pong across the 8 PSUM banks so evictions (bias-add on DVE, bf16 out)
and stores overlap the next group's matmuls.  Junk columns w in {56,57} of
each 58-wide output row are stripped on the host.
"""

import numpy as np
import ml_dtypes

import concourse.bacc as bacc
import concourse.tile as tile
import concourse.mybir as mybir
from concourse.bass_utils import run_bass_kernel_spmd

F32 = mybir.dt.float32
BF16 = mybir.dt.bfloat16
BF = ml_dtypes.bfloat16

B, CIN, COUT, H, W, K = 32, 256, 256, 56, 56, 3
NCORES = 8
BPC = B // NCORES          # images per core
WP = W + 2                 # padded row width (58)
HP = H + 3                 # padded rows (59): 1 top, 2 bottom (tail tap reads)
XF = HP * WP               # padded flat length per image-channel (3422)
OF = H * WP                # padded output flat length (3248)
NT = 7                     # output tiles per (img, cout-chunk)
NFREE = OF // NT           # 464 positions per matmul
XLOAD = 3366               # matmuls never read past 3365
NW = K * K * 128           # weight free length per (ci, cc): 9 taps x 128 couts

_CACHE = {}


def _build():
    if "nc" in _CACHE:
        return _CACHE["nc"]
    nc = bacc.Bacc("TRN2", target_bir_lowering=False, debug=False,
                   num_swdge_queues=1)
    x_d = nc.dram_tensor("x", [BPC, CIN, XF], BF16, kind="ExternalInput").ap()
    w_d = nc.dram_tensor("w", [2, 128, 2, NW], BF16, kind="ExternalInput").ap()
    b_d = nc.dram_tensor("b", [COUT], F32, kind="ExternalInput").ap()
    o_d = nc.dram_tensor("o", [BPC, COUT, OF], BF16, kind="ExternalOutput").ap()

    with tile.TileContext(nc) as tc:
        with (
            tc.tile_pool(name="wp", bufs=1) as wp,
            tc.tile_pool(name="xp", bufs=6) as xp,
            tc.tile_pool(name="op", bufs=2) as op,
            tc.tile_pool(name="pp", bufs=8, space="PSUM") as pp,
        ):
            # DMA trigger instructions cost ~0.7us EACH on the issuing
            # engine, so issue in parallel from both HWDGE engines:
            # sync carries ci=0 traffic, scalar carries ci=1.
            eng = [nc.sync, nc.scalar]

            bias_t = wp.tile([128, 2], F32)
            # weights [cin-in-chunk, ci, cc, tap*128+cout]: one contiguous
            # 295KB DMA per (ci, cc) chunk.
            w_t = wp.tile([128, 2, 2, NW], BF16)

            def w_dma(e, ci, cc, lo=0, hi=K * K):
                e.dma_start(out=w_t[:, ci, cc, lo * 128:hi * 128],
                            in_=w_d[ci, :, cc, lo * 128:hi * 128])

            def x_dma(e, xs, img, ci, lo, hi):
                e.dma_start(
                    out=xs[ci][:, lo:hi],
                    in_=x_d[img, ci * 128:(ci + 1) * 128, lo:hi],
                )

            # steady images: 4 coarse slices (they prefetch a whole image
            # ahead anyway)
            xsl = [0, 846, 1692, 2538, XLOAD]

            def load_img0():
                # Hand-scheduled by need-by time: mi walks ci0 taps 0-8
                # (~0.78us each) then ci1, so ci=1 data has ~7us of slack —
                # the scalar engine fronts ci0's mid slice while sync lands
                # the critical x[0:640]+w taps that gate the first matmul.
                xs = [xp.tile([128, XF], BF16, tag="x", name=f"x_0_{ci}")
                      for ci in range(2)]
                s0, s1 = eng

                def X(e, ci, a, b):
                    x_dma(e, xs, 0, ci, a, b)

                X(s0, 0, 0, 292); w_dma(s0, 0, 0, 0, 3)
                X(s0, 0, 292, 640); X(s0, 0, 640, 1046)
                w_dma(s0, 0, 0, 3, K * K)
                X(s0, 0, 1974, 2902)
                s0.dma_start(out=bias_t[:, 0:1], in_=b_d[0:128])
                w_dma(s0, 0, 1)

                X(s1, 0, 1046, 1974)
                X(s1, 1, 0, 292); w_dma(s1, 1, 0, 0, 3)
                X(s1, 1, 292, 640); X(s1, 1, 640, 1046)
                w_dma(s1, 1, 0, 3, K * K)
                X(s1, 1, 1046, 1974); X(s1, 0, 2902, XLOAD)
                X(s1, 1, 1974, 2902); X(s1, 1, 2902, XLOAD)
                s1.dma_start(out=bias_t[:, 1:2], in_=b_d[128:256])
                w_dma(s1, 1, 1)
                return xs

            def load_img(img, first=False):
                if first:
                    return load_img0()
                xs = [xp.tile([128, XF], BF16, tag="x", name=f"x_{img}_{ci}")
                      for ci in range(2)]
                for ci in range(2):
                    e = eng[ci]
                    for s in range(len(xsl) - 1):
                        x_dma(e, xs, img, ci, xsl[s], xsl[s + 1])
                return xs

            def do_pass(xs, cc, o_t, img, nts, fine=False):
                """One PSUM accumulation wave over banks `nts`: each of the
                18 (ci, tap) weight tiles is LDWEIGHTS'd once and streamed
                through len(nts) matmuls (ldweights=False on all of them)."""
                pss = [pp.tile([128, NFREE], F32, tag="ps",
                               name=f"ps_{img}_{cc}_{nt}") for nt in nts]
                for mi, (ci, t) in enumerate(
                    [(ci, t) for ci in range(2) for t in range(K * K)]
                ):
                    kh, kw = divmod(t, K)
                    wsl = w_t[:, ci, cc, t * 128:(t + 1) * 128]
                    for ps, nt in zip(pss, nts):
                        off = nt * NFREE + kh * WP + kw
                        nc.tensor.matmul(
                            ps, wsl, xs[ci][:, off:off + NFREE],
                            start=(mi == 0), stop=(mi == 17),
                        )
                # bias-add + PSUM eviction on the otherwise-idle DVE,
                # bf16 out halves store traffic
                for j, (ps, nt) in enumerate(zip(pss, nts)):
                    nc.vector.tensor_scalar_add(
                        o_t[:, nt * NFREE:(nt + 1) * NFREE],
                        ps,
                        bias_t[:, cc:cc + 1],
                    )
                    if fine:
                        eng[j % 2].dma_start(
                            out=o_d[img, cc * 128:(cc + 1) * 128,
                                    nt * NFREE:(nt + 1) * NFREE],
                            in_=o_t[:, nt * NFREE:(nt + 1) * NFREE],
                        )
                if not fine:
                    lo, hi = nts[0] * NFREE, (nts[-1] + 1) * NFREE
                    eng[nts[0] % 2].dma_start(
                        out=o_d[img, cc * 128:(cc + 1) * 128, lo:hi],
                        in_=o_t[:, lo:hi],
                    )

            # warm the PE clock (p-state ramps over ~3us of activity) with
            # dummy matmuls on a memset tile while the first DMAs land
            wu = wp.tile([128, NFREE], BF16)
            nc.gpsimd.memset(wu, 0)
            ps_warm = pp.tile([128, NFREE], F32, tag="ps", name="ps_warm")
            for _ in range(6):
                nc.tensor.matmul(ps_warm, wu[:, 0:128], wu,
                                 start=True, stop=True)

            for img in range(BPC):
                xs = load_img(img, first=(img == 0))
                for cc in range(2):
                    o_t = op.tile([128, OF], BF16, tag="o")
                    last = img == BPC - 1 and cc == 1
                    if last:
                        # taper the final passes so the drain tail is short
                        do_pass(xs, cc, o_t, img, [0, 1, 2, 3])
                        do_pass(xs, cc, o_t, img, [4, 5], fine=True)
                        do_pass(xs, cc, o_t, img, [6], fine=True)
                    else:
                        do_pass(xs, cc, o_t, img, [0, 1, 2, 3])
                        do_pass(xs, cc, o_t, img, [4, 5, 6])
    nc.compile()
    _CACHE["nc"] = nc
    return nc


def make_in_maps(inp, kernel, bias):
    xpad = np.zeros((B, CIN, HP, WP), np.float32)
    xpad[:, :, 1:1 + H, 1:1 + W] = inp
    xflat = xpad.reshape(B, CIN, XF).astype(BF)
    # [cout, cin, kh, kw] -> [ci, cin_in, cc, tap*128+cout_in]
    kk = np.asarray(kernel, np.float32).reshape(2, 128, 2, 128, K, K)
    w_dev = np.ascontiguousarray(
        kk.transpose(2, 3, 0, 4, 5, 1).reshape(2, 128, 2, NW)
    ).astype(BF)
    b_dev = np.ascontiguousarray(np.asarray(bias, np.float32))
    return [
        {"x": np.ascontiguousarray(xflat[c * BPC:(c + 1) * BPC]),
         "w": w_dev, "b": b_dev}
        for c in range(NCORES)
    ]


def assemble(results):
    o = np.concatenate([np.asarray(results[c]["o"]) for c in range(NCORES)],
                       axis=0)
    return np.ascontiguousarray(
        o.reshape(B, COUT, H, WP)[:, :, :, :W].astype(np.float32)
    )


def kernel(inp, kernel, bias):
    nc = _build()
    in_maps = make_in_maps(inp, kernel, bias)
    r = run_bass_kernel_spmd(nc, in_maps, core_ids=list(range(NCORES)))
    return assemble(r.results)


# revision 16
# speedup vs baseline: 1.3242x; 1.0130x over previous
"""Trainium2 Bass kernel for nn_Conv2d (B=32, 256->256, 56x56, 3x3, pad=1) + bias.

Strategy
--------
Data-parallel over batch: 4 images per NeuronCore x 8 cores; weights/bias
replicated; no collectives.

Per core, the conv is computed as shifted matmuls on a host-padded input
(59 rows x 58 cols per image-channel): the tap (kh, kw) contribution to
output row-block [8n, 8n+8) is a matmul whose moving operand is the 2D
window x[:, 8n+kh : 8n+kh+8, kw : kw+56] (row stride 58) and whose PSUM
output is [8, 56] contiguous — so no junk columns are ever computed.
Operands are bf16 (rel err ~4e-3 vs the 2e-2 gate): bf16 self-loading
matmuls hide their LDWEIGHTS under the previous matmul's stream, unlike
f32r.  Each (img, cout-chunk) accumulates 18 (cin-chunk, tap) matmuls per
PSUM bank; banks are grouped 4+3 and ping-pong across the 8 PSUM banks so
evictions (bias-add on DVE, bf16 out) and stores overlap the next group's
matmuls.  Dummy matmuls on a memset tile warm the PE clock (p-state) while
the first DMAs land (~5us fixed DMA-engine startup latency).
"""

import numpy as np
import ml_dtypes

import concourse.bacc as bacc
import concourse.tile as tile
import concourse.mybir as mybir
from concourse.bass_utils import run_bass_kernel_spmd

F32 = mybir.dt.float32
BF16 = mybir.dt.bfloat16
BF = ml_dtypes.bfloat16

B, CIN, COUT, H, W, K = 32, 256, 256, 56, 56, 3
NCORES = 8
BPC = B // NCORES          # images per core
WP = W + 2                 # padded row width (58)
HP = H + 3                 # padded rows (59): 1 top, 2 bottom (tail tap reads)
NT = 7                     # output row-blocks per (img, cout-chunk)
RB = H // NT               # 8 output rows per block
NFREE = RB * W             # 448 output positions per matmul
NW = K * K * 128           # weight free length per (ci, cc): 9 taps x 128 couts

_CACHE = {}


def _build():
    if "nc" in _CACHE:
        return _CACHE["nc"]
    nc = bacc.Bacc("TRN2", target_bir_lowering=False, debug=False,
                   num_swdge_queues=1)
    x_d = nc.dram_tensor("x", [BPC, CIN, HP, WP], BF16,
                         kind="ExternalInput").ap()
    w_d = nc.dram_tensor("w", [2, 128, 2, NW], BF16, kind="ExternalInput").ap()
    b_d = nc.dram_tensor("b", [COUT], F32, kind="ExternalInput").ap()
    o_d = nc.dram_tensor("o", [BPC, COUT, H, W], BF16,
                         kind="ExternalOutput").ap()

    with tile.TileContext(nc) as tc:
        with (
            tc.tile_pool(name="wp", bufs=1) as wp,
            tc.tile_pool(name="xp", bufs=6) as xp,
            tc.tile_pool(name="op", bufs=2) as op,
            tc.tile_pool(name="pp", bufs=8, space="PSUM") as pp,
        ):
            # DMA trigger instructions cost ~0.65us EACH on the issuing
            # engine, so issue in parallel from both HWDGE engines:
            # sync carries ci=0 traffic, scalar carries ci=1.
            eng = [nc.sync, nc.scalar]

            bias_t = wp.tile([128, 2], F32)
            # weights [cin-in-chunk, ci, cc, tap*128+cout]: one contiguous
            # 295KB DMA per (ci, cc) chunk, split 3+6 taps for cc=0.
            w_t = wp.tile([128, 2, 2, NW], BF16)

            def w_dma(e, ci, cc, lo=0, hi=K * K):
                e.dma_start(out=w_t[:, ci, cc, lo * 128:hi * 128],
                            in_=w_d[ci, :, cc, lo * 128:hi * 128])

            def x_dma(e, xs, img, ci, lo, hi):
                e.dma_start(
                    out=xs[ci][:, lo:hi, :],
                    in_=x_d[img, ci * 128:(ci + 1) * 128, lo:hi, :],
                )

            # steady images: 4 coarse row-slices (they prefetch a whole
            # image ahead anyway)
            xsl = [0, 15, 30, 45, HP]

            def load_img0():
                # Hand-scheduled by need-by time: mi walks ci0 taps 0-8
                # (~0.75us each) then ci1, so ci=1 data has ~7us of slack —
                # the scalar engine fronts ci0's mid rows while sync lands
                # the critical rows [0:8)+w taps that gate the first matmul.
                xs = [xp.tile([128, HP, WP], BF16, tag="x", name=f"x_0_{ci}")
                      for ci in range(2)]
                s0, s1 = eng

                def X(e, ci, a, b):
                    x_dma(e, xs, 0, ci, a, b)

                X(s0, 0, 0, 8); w_dma(s0, 0, 0, 0, 3)
                X(s0, 0, 8, 18); X(s0, 0, 18, 27)
                w_dma(s0, 0, 0, 3, K * K)
                X(s0, 0, 27, 35); X(s0, 0, 44, HP)
                s0.dma_start(out=bias_t[:, 0:1], in_=b_d[0:128])
                w_dma(s0, 0, 1)

                X(s1, 0, 35, 44)
                X(s1, 1, 0, 8); w_dma(s1, 1, 0, 0, 3)
                X(s1, 1, 8, 18); X(s1, 1, 18, 27)
                w_dma(s1, 1, 0, 3, K * K)
                X(s1, 1, 27, 44); X(s1, 1, 44, HP)
                s1.dma_start(out=bias_t[:, 1:2], in_=b_d[128:256])
                w_dma(s1, 1, 1)
                return xs

            def load_img(img, first=False):
                if first:
                    return load_img0()
                xs = [xp.tile([128, HP, WP], BF16, tag="x",
                              name=f"x_{img}_{ci}") for ci in range(2)]
                for ci in range(2):
                    e = eng[ci]
                    for s in range(len(xsl) - 1):
                        x_dma(e, xs, img, ci, xsl[s], xsl[s + 1])
                return xs

            def do_pass(xs, cc, o_t, img, nts, fine=False):
                """One PSUM accumulation wave over row-blocks `nts`: 18
                (ci, tap) x len(nts) matmuls, weight-outermost so the bf16
                self-load LDWEIGHTS amortizes over len(nts) streams."""
                pss = [pp.tile([128, RB, W], F32, tag="ps",
                               name=f"ps_{img}_{cc}_{nt}") for nt in nts]
                for mi, (ci, t) in enumerate(
                    [(ci, t) for ci in range(2) for t in range(K * K)]
                ):
                    kh, kw = divmod(t, K)
                    wsl = w_t[:, ci, cc, t * 128:(t + 1) * 128]
                    for ps, nt in zip(pss, nts):
                        r0 = nt * RB + kh
                        nc.tensor.matmul(
                            ps, wsl, xs[ci][:, r0:r0 + RB, kw:kw + W],
                            start=(mi == 0), stop=(mi == 17),
                        )
                # bias-add + PSUM eviction on the otherwise-idle DVE,
                # bf16 out halves store traffic
                for j, (ps, nt) in enumerate(zip(pss, nts)):
                    nc.vector.tensor_scalar_add(
                        o_t[:, nt * RB:(nt + 1) * RB, :],
                        ps,
                        bias_t[:, cc:cc + 1],
                    )
                    if fine:
                        eng[j % 2].dma_start(
                            out=o_d[img, cc * 128:(cc + 1) * 128,
                                    nt * RB:(nt + 1) * RB, :],
                            in_=o_t[:, nt * RB:(nt + 1) * RB, :],
                        )
                if not fine:
                    lo, hi = nts[0] * RB, (nts[-1] + 1) * RB
                    eng[nts[0] % 2].dma_start(
                        out=o_d[img, cc * 128:(cc + 1) * 128, lo:hi, :],
                        in_=o_t[:, lo:hi, :],
                    )

            # warm the PE clock (p-state ramps over ~9us of activity) with
            # dummy matmuls on a memset tile while the first DMAs land
            # (~5.5us trigger-to-completion on a cold DMA engine)
            wu = wp.tile([128, NFREE], BF16)
            nc.gpsimd.memset(wu, 0)
            ps_warm = pp.tile([128, NFREE], F32, tag="ps", name="ps_warm")
            for _ in range(11):
                nc.tensor.matmul(ps_warm, wu[:, 0:128], wu,
                                 start=True, stop=True)

            for img in range(BPC):
                xs = load_img(img, first=(img == 0))
                for cc in range(2):
                    o_t = op.tile([128, H, W], BF16, tag="o",
                                  name=f"o_{img}_{cc}")
                    last = img == BPC - 1 and cc == 1
                    if last:
                        # taper the final passes so the drain tail is short
                        do_pass(xs, cc, o_t, img, [0, 1, 2, 3])
                        do_pass(xs, cc, o_t, img, [4, 5], fine=True)
                        do_pass(xs, cc, o_t, img, [6], fine=True)
                    else:
                        do_pass(xs, cc, o_t, img, [0, 1, 2, 3])
                        do_pass(xs, cc, o_t, img, [4, 5, 6])
    nc.compile()
    _CACHE["nc"] = nc
    return nc


def make_in_maps(inp, kernel, bias):
    xpad = np.zeros((B, CIN, HP, WP), np.float32)
    xpad[:, :, 1:1 + H, 1:1 + W] = inp
    xdev = xpad.astype(BF)
    # [cout, cin, kh, kw] -> [ci, cin_in, cc, tap*128+cout_in]
    kk = np.asarray(kernel, np.float32).reshape(2, 128, 2, 128, K, K)
    w_dev = np.ascontiguousarray(
        kk.transpose(2, 3, 0, 4, 5, 1).reshape(2, 128, 2, NW)
    ).astype(BF)
    b_dev = np.ascontiguousarray(np.asarray(bias, np.float32))
    return [
        {"x": np.ascontiguousarray(xdev[c * BPC:(c + 1) * BPC]),
         "w": w_dev, "b": b_dev}
        for c in range(NCORES)
    ]


def assemble(results):
    o = np.concatenate([np.asarray(results[c]["o"]) for c in range(NCORES)],
                       axis=0)
    return np.ascontiguousarray(o.astype(np.float32))


def kernel(inp, kernel, bias):
    nc = _build()
    in_maps = make_in_maps(inp, kernel, bias)
    r = run_bass_kernel_spmd(nc, in_maps, core_ids=list(range(NCORES)))
    return assemble(r.results)


# revision 18
# speedup vs baseline: 1.3489x; 1.0186x over previous
"""Trainium2 Bass kernel for nn_Conv2d (B=32, 256->256, 56x56, 3x3, pad=1) + bias.

Strategy
--------
Data-parallel over batch: 4 images per NeuronCore x 8 cores; weights/bias
replicated; no collectives.

Per core, the conv is computed as shifted matmuls on a host-padded input
(59 rows x 58 cols per image-channel): the tap (kh, kw) contribution to
output row-block [8n, 8n+8) is a matmul whose moving operand is the 2D
window x[:, 8n+kh : 8n+kh+8, kw : kw+56] (row stride 58) and whose PSUM
output is [8, 56] contiguous — so no junk columns are ever computed.
Operands are bf16 (rel err ~4e-3 vs the 2e-2 gate): bf16 self-loading
matmuls hide their LDWEIGHTS under the previous matmul's stream, unlike
f32r.  Each (img, cout-chunk) accumulates 18 (cin-chunk, tap) matmuls per
PSUM bank; banks are grouped 4+3 and ping-pong across the 8 PSUM banks so
evictions (bias-add on DVE, bf16 out) and stores overlap the next group's
matmuls.  Dummy matmuls on a memset tile warm the PE clock (p-state) while
the first DMAs land (~5us fixed DMA-engine startup latency).
"""

import numpy as np
import ml_dtypes

import concourse.bacc as bacc
import concourse.tile as tile
import concourse.mybir as mybir
from concourse.bass_utils import run_bass_kernel_spmd

F32 = mybir.dt.float32
BF16 = mybir.dt.bfloat16
BF = ml_dtypes.bfloat16

B, CIN, COUT, H, W, K = 32, 256, 256, 56, 56, 3
NCORES = 8
BPC = B // NCORES          # images per core
WP = W + 2                 # padded row width (58)
HP = H + 3                 # padded rows (59): 1 top, 2 bottom (tail tap reads)
NT = 7                     # output row-blocks per (img, cout-chunk)
RB = H // NT               # 8 output rows per block
NFREE = RB * W             # 448 output positions per matmul
NW = K * K * 128           # weight free length per (ci, cc): 9 taps x 128 couts

_CACHE = {}


def _build():
    if "nc" in _CACHE:
        return _CACHE["nc"]
    nc = bacc.Bacc("TRN2", target_bir_lowering=False, debug=False,
                   num_swdge_queues=1)
    x_d = nc.dram_tensor("x", [BPC, CIN, HP, WP], BF16,
                         kind="ExternalInput").ap()
    w_d = nc.dram_tensor("w", [2, 128, 2, NW], BF16, kind="ExternalInput").ap()
    b_d = nc.dram_tensor("b", [COUT], F32, kind="ExternalInput").ap()
    o_d = nc.dram_tensor("o", [BPC, COUT, H, W], BF16,
                         kind="ExternalOutput").ap()

    with tile.TileContext(nc) as tc:
        with (
            tc.tile_pool(name="wp", bufs=1) as wp,
            tc.tile_pool(name="xp", bufs=6) as xp,
            tc.tile_pool(name="op", bufs=2) as op,
            tc.tile_pool(name="pp", bufs=8, space="PSUM") as pp,
        ):
            # DMA trigger instructions cost ~0.65us EACH on the issuing
            # engine, so issue in parallel from both HWDGE engines:
            # sync carries ci=0 traffic, scalar carries ci=1.
            eng = [nc.sync, nc.scalar]

            bias_t = wp.tile([128, 2], F32)
            # weights [cin-in-chunk, ci, cc, tap*128+cout]: one contiguous
            # 295KB DMA per (ci, cc) chunk, split 3+6 taps for cc=0.
            w_t = wp.tile([128, 2, 2, NW], BF16)

            def w_dma(e, ci, cc, lo=0, hi=K * K):
                e.dma_start(out=w_t[:, ci, cc, lo * 128:hi * 128],
                            in_=w_d[ci, :, cc, lo * 128:hi * 128])

            def x_dma(e, xs, img, ci, lo, hi):
                e.dma_start(
                    out=xs[ci][:, lo:hi, :],
                    in_=x_d[img, ci * 128:(ci + 1) * 128, lo:hi, :],
                )

            # steady images: 4 coarse row-slices (they prefetch a whole
            # image ahead anyway)
            xsl = [0, 15, 30, 45, HP]

            def load_img0():
                # Hand-scheduled by need-by time: mi walks ci0 taps 0-8
                # (~0.75us each) then ci1, so ci=1 data has ~7us of slack —
                # the scalar engine fronts ci0's mid rows while sync lands
                # the critical rows [0:8)+w taps that gate the first matmul.
                xs = [xp.tile([128, HP, WP], BF16, tag="x", name=f"x_0_{ci}")
                      for ci in range(2)]
                s0, s1 = eng

                def X(e, ci, a, b):
                    x_dma(e, xs, 0, ci, a, b)

                # the first matmul wave (4 row-blocks) touches ci0 rows
                # 0-33 within ~1us, so scalar fronts rows 18-35 while sync
                # lands rows 0-18 + the gating weight taps
                X(s0, 0, 0, 8); w_dma(s0, 0, 0, 0, 3)
                X(s0, 0, 8, 18)
                w_dma(s0, 0, 0, 3, K * K)
                X(s0, 0, 35, 44); X(s0, 0, 44, HP)
                s0.dma_start(out=bias_t[:, 0:1], in_=b_d[0:128])
                w_dma(s0, 0, 1)

                X(s1, 0, 18, 35)
                X(s1, 1, 0, 8); w_dma(s1, 1, 0, 0, 3)
                X(s1, 1, 8, 18); X(s1, 1, 18, 27)
                w_dma(s1, 1, 0, 3, K * K)
                X(s1, 1, 27, 44); X(s1, 1, 44, HP)
                s1.dma_start(out=bias_t[:, 1:2], in_=b_d[128:256])
                w_dma(s1, 1, 1)
                return xs

            def load_img(img, first=False):
                if first:
                    return load_img0()
                xs = [xp.tile([128, HP, WP], BF16, tag="x",
                              name=f"x_{img}_{ci}") for ci in range(2)]
                for ci in range(2):
                    e = eng[ci]
                    for s in range(len(xsl) - 1):
                        x_dma(e, xs, img, ci, xsl[s], xsl[s + 1])
                return xs

            def do_pass(xs, cc, o_t, img, nts, fine=False):
                """One PSUM accumulation wave over row-blocks `nts`: 18
                (ci, tap) x len(nts) matmuls, weight-outermost so the bf16
                self-load LDWEIGHTS amortizes over len(nts) streams."""
                pss = [pp.tile([128, RB, W], F32, tag="ps",
                               name=f"ps_{img}_{cc}_{nt}") for nt in nts]
                for mi, (ci, t) in enumerate(
                    [(ci, t) for ci in range(2) for t in range(K * K)]
                ):
                    kh, kw = divmod(t, K)
                    wsl = w_t[:, ci, cc, t * 128:(t + 1) * 128]
                    for ps, nt in zip(pss, nts):
                        r0 = nt * RB + kh
                        nc.tensor.matmul(
                            ps, wsl, xs[ci][:, r0:r0 + RB, kw:kw + W],
                            start=(mi == 0), stop=(mi == 17),
                        )
                # bias-add + PSUM eviction on the otherwise-idle DVE,
                # bf16 out halves store traffic
                for j, (ps, nt) in enumerate(zip(pss, nts)):
                    if fine and len(nts) == 1:
                        # final bank: half-row-block evicts + stores on both
                        # engines to shorten the drain tail
                        hb = RB // 2
                        for h in range(2):
                            r = nt * RB + h * hb
                            nc.vector.tensor_scalar_add(
                                o_t[:, r:r + hb, :],
                                ps[:, h * hb:(h + 1) * hb, :],
                                bias_t[:, cc:cc + 1],
                            )
                            eng[h].dma_start(
                                out=o_d[img, cc * 128:(cc + 1) * 128,
                                        r:r + hb, :],
                                in_=o_t[:, r:r + hb, :],
                            )
                        continue
                    nc.vector.tensor_scalar_add(
                        o_t[:, nt * RB:(nt + 1) * RB, :],
                        ps,
                        bias_t[:, cc:cc + 1],
                    )
                    if fine:
                        eng[j % 2].dma_start(
                            out=o_d[img, cc * 128:(cc + 1) * 128,
                                    nt * RB:(nt + 1) * RB, :],
                            in_=o_t[:, nt * RB:(nt + 1) * RB, :],
                        )
                if not fine:
                    lo, hi = nts[0] * RB, (nts[-1] + 1) * RB
                    eng[nts[0] % 2].dma_start(
                        out=o_d[img, cc * 128:(cc + 1) * 128, lo:hi, :],
                        in_=o_t[:, lo:hi, :],
                    )

            # warm the PE clock (p-state ramps over ~9us of activity) with
            # dummy matmuls on a memset tile while the first DMAs land
            # (~5.5us trigger-to-completion on a cold DMA engine)
            wu = wp.tile([128, NFREE], BF16)
            nc.gpsimd.memset(wu, 0)
            ps_warm = pp.tile([128, NFREE], F32, tag="ps", name="ps_warm")
            for _ in range(11):
                nc.tensor.matmul(ps_warm, wu[:, 0:128], wu,
                                 start=True, stop=True)

            for img in range(BPC):
                xs = load_img(img, first=(img == 0))
                for cc in range(2):
                    o_t = op.tile([128, H, W], BF16, tag="o",
                                  name=f"o_{img}_{cc}")
                    last = img == BPC - 1 and cc == 1
                    if last:
                        # taper the final passes so the drain tail is short
                        do_pass(xs, cc, o_t, img, [0, 1, 2, 3])
                        do_pass(xs, cc, o_t, img, [4, 5], fine=True)
                        do_pass(xs, cc, o_t, img, [6], fine=True)
                    else:
                        do_pass(xs, cc, o_t, img, [0, 1, 2, 3])
                        do_pass(xs, cc, o_t, img, [4, 5, 6])
    nc.compile()
    _CACHE["nc"] = nc
    return nc


def make_in_maps(inp, kernel, bias):
    xpad = np.zeros((B, CIN, HP, WP), np.float32)
    xpad[:, :, 1:1 + H, 1:1 + W] = inp
    xdev = xpad.astype(BF)
    # [cout, cin, kh, kw] -> [ci, cin_in, cc, tap*128+cout_in]
    kk = np.asarray(kernel, np.float32).reshape(2, 128, 2, 128, K, K)
    w_dev = np.ascontiguousarray(
        kk.transpose(2, 3, 0, 4, 5, 1).reshape(2, 128, 2, NW)
    ).astype(BF)
    b_dev = np.ascontiguousarray(np.asarray(bias, np.float32))
    return [
        {"x": np.ascontiguousarray(xdev[c * BPC:(c + 1) * BPC]),
         "w": w_dev, "b": b_dev}
        for c in range(NCORES)
    ]


def assemble(results):
    o = np.concatenate([np.asarray(results[c]["o"]) for c in range(NCORES)],
                       axis=0)
    return np.ascontiguousarray(o.astype(np.float32))


def kernel(inp, kernel, bias):
    nc = _build()
    in_maps = make_in_maps(inp, kernel, bias)
    r = run_bass_kernel_spmd(nc, in_maps, core_ids=list(range(NCORES)))
    return assemble(r.results)


# revision 20
# speedup vs baseline: 1.3498x; 1.0007x over previous
"""Trainium2 Bass kernel for nn_Conv2d (B=32, 256->256, 56x56, 3x3, pad=1) + bias.

Strategy
--------
Data-parallel over batch: 4 images per NeuronCore x 8 cores; weights/bias
replicated; no collectives.

Per core, the conv is computed as shifted matmuls on a host-padded input
(59 rows x 58 cols per image-channel): the tap (kh, kw) contribution to
output row-block [8n, 8n+8) is a matmul whose moving operand is the 2D
window x[:, 8n+kh : 8n+kh+8, kw : kw+56] (row stride 58) and whose PSUM
output is [8, 56] contiguous — so no junk columns are ever computed.
Operands are bf16 (rel err ~4e-3 vs the 2e-2 gate): bf16 self-loading
matmuls hide their LDWEIGHTS under the previous matmul's stream, unlike
f32r.  Each (img, cout-chunk) accumulates 18 (cin-chunk, tap) matmuls per
PSUM bank; banks are grouped 4+3 and ping-pong across the 8 PSUM banks so
evictions (bias-add on DVE, bf16 out) and stores overlap the next group's
matmuls.  Dummy matmuls on a memset tile warm the PE clock (p-state) while
the first DMAs land (~5us fixed DMA-engine startup latency).
"""

import numpy as np
import ml_dtypes

import concourse.bacc as bacc
import concourse.tile as tile
import concourse.mybir as mybir
from concourse.bass_utils import run_bass_kernel_spmd

F32 = mybir.dt.float32
BF16 = mybir.dt.bfloat16
BF = ml_dtypes.bfloat16

B, CIN, COUT, H, W, K = 32, 256, 256, 56, 56, 3
NCORES = 8
BPC = B // NCORES          # images per core
WP = W + 2                 # padded row width (58)
HP = H + 3                 # padded rows (59): 1 top, 2 bottom (tail tap reads)
NT = 7                     # output row-blocks per (img, cout-chunk)
RB = H // NT               # 8 output rows per block
NFREE = RB * W             # 448 output positions per matmul
NW = K * K * 128           # weight free length per (ci, cc): 9 taps x 128 couts

_CACHE = {}


def _build():
    if "nc" in _CACHE:
        return _CACHE["nc"]
    nc = bacc.Bacc("TRN2", target_bir_lowering=False, debug=False,
                   num_swdge_queues=1)
    x_d = nc.dram_tensor("x", [BPC, CIN, HP, WP], BF16,
                         kind="ExternalInput").ap()
    w_d = nc.dram_tensor("w", [2, 128, 2, NW], BF16, kind="ExternalInput").ap()
    b_d = nc.dram_tensor("b", [COUT], F32, kind="ExternalInput").ap()
    o_d = nc.dram_tensor("o", [BPC, COUT, H, W], BF16,
                         kind="ExternalOutput").ap()

    with tile.TileContext(nc) as tc:
        with (
            tc.tile_pool(name="wp", bufs=1) as wp,
            tc.tile_pool(name="xp", bufs=6) as xp,
            tc.tile_pool(name="op", bufs=2) as op,
            tc.tile_pool(name="pp", bufs=8, space="PSUM") as pp,
        ):
            # DMA trigger instructions cost ~0.65us EACH on the issuing
            # engine, so issue in parallel from both HWDGE engines:
            # sync carries ci=0 traffic, scalar carries ci=1.
            eng = [nc.sync, nc.scalar]

            bias_t = wp.tile([128, 2], F32)
            # weights [cin-in-chunk, ci, cc, tap*128+cout]: one contiguous
            # 295KB DMA per (ci, cc) chunk, split 3+6 taps for cc=0.
            w_t = wp.tile([128, 2, 2, NW], BF16)

            def w_dma(e, ci, cc, lo=0, hi=K * K):
                e.dma_start(out=w_t[:, ci, cc, lo * 128:hi * 128],
                            in_=w_d[ci, :, cc, lo * 128:hi * 128])

            def x_dma(e, xs, img, ci, lo, hi):
                e.dma_start(
                    out=xs[ci][:, lo:hi, :],
                    in_=x_d[img, ci * 128:(ci + 1) * 128, lo:hi, :],
                )

            # steady images: 4 coarse row-slices (they prefetch a whole
            # image ahead anyway)
            xsl = [0, 15, 30, 45, HP]

            def load_img0():
                # Hand-scheduled by need-by time: mi walks ci0 taps 0-8
                # (~0.75us each) then ci1, so ci=1 data has ~7us of slack —
                # the scalar engine fronts ci0's mid rows while sync lands
                # the critical rows [0:8)+w taps that gate the first matmul.
                xs = [xp.tile([128, HP, WP], BF16, tag="x", name=f"x_0_{ci}")
                      for ci in range(2)]
                s0, s1 = eng

                def X(e, ci, a, b):
                    x_dma(e, xs, 0, ci, a, b)

                # the first matmul wave (4 row-blocks) touches ci0 rows
                # 0-33 within ~1us, so scalar fronts rows 18-35 while sync
                # lands rows 0-18 + the gating weight taps
                X(s0, 0, 0, 8); w_dma(s0, 0, 0, 0, 1)
                X(s0, 0, 8, 18); w_dma(s0, 0, 0, 1, 3)
                w_dma(s0, 0, 0, 3, K * K)
                X(s0, 0, 35, 44); X(s0, 0, 44, HP)
                s0.dma_start(out=bias_t[:, 0:1], in_=b_d[0:128])
                w_dma(s0, 0, 1)

                X(s1, 0, 18, 27); X(s1, 0, 27, 35)
                X(s1, 1, 0, 8); w_dma(s1, 1, 0, 0, 3)
                X(s1, 1, 8, 18); X(s1, 1, 18, 27)
                w_dma(s1, 1, 0, 3, K * K)
                X(s1, 1, 27, 44); X(s1, 1, 44, HP)
                s1.dma_start(out=bias_t[:, 1:2], in_=b_d[128:256])
                w_dma(s1, 1, 1)
                return xs

            def load_img(img, first=False):
                if first:
                    return load_img0()
                xs = [xp.tile([128, HP, WP], BF16, tag="x",
                              name=f"x_{img}_{ci}") for ci in range(2)]
                for ci in range(2):
                    e = eng[ci]
                    for s in range(len(xsl) - 1):
                        x_dma(e, xs, img, ci, xsl[s], xsl[s + 1])
                return xs

            def do_pass(xs, cc, o_t, img, nts, fine=False):
                """One PSUM accumulation wave over row-blocks `nts`: 18
                (ci, tap) x len(nts) matmuls, weight-outermost so the bf16
                self-load LDWEIGHTS amortizes over len(nts) streams."""
                pss = [pp.tile([128, RB, W], F32, tag="ps",
                               name=f"ps_{img}_{cc}_{nt}") for nt in nts]
                for mi, (ci, t) in enumerate(
                    [(ci, t) for ci in range(2) for t in range(K * K)]
                ):
                    kh, kw = divmod(t, K)
                    wsl = w_t[:, ci, cc, t * 128:(t + 1) * 128]
                    for ps, nt in zip(pss, nts):
                        r0 = nt * RB + kh
                        nc.tensor.matmul(
                            ps, wsl, xs[ci][:, r0:r0 + RB, kw:kw + W],
                            start=(mi == 0), stop=(mi == 17),
                        )
                # bias-add + PSUM eviction on the otherwise-idle DVE,
                # bf16 out halves store traffic
                for j, (ps, nt) in enumerate(zip(pss, nts)):
                    if fine and len(nts) == 1:
                        # final bank: half-row-block evicts + stores on both
                        # engines to shorten the drain tail
                        hb = RB // 2
                        for h in range(2):
                            r = nt * RB + h * hb
                            nc.vector.tensor_scalar_add(
                                o_t[:, r:r + hb, :],
                                ps[:, h * hb:(h + 1) * hb, :],
                                bias_t[:, cc:cc + 1],
                            )
                            eng[h].dma_start(
                                out=o_d[img, cc * 128:(cc + 1) * 128,
                                        r:r + hb, :],
                                in_=o_t[:, r:r + hb, :],
                            )
                        continue
                    nc.vector.tensor_scalar_add(
                        o_t[:, nt * RB:(nt + 1) * RB, :],
                        ps,
                        bias_t[:, cc:cc + 1],
                    )
                    if fine:
                        eng[j % 2].dma_start(
                            out=o_d[img, cc * 128:(cc + 1) * 128,
                                    nt * RB:(nt + 1) * RB, :],
                            in_=o_t[:, nt * RB:(nt + 1) * RB, :],
                        )
                if not fine:
                    # region A -> sync, region B -> scalar: keeps either
                    # HWDGE queue from accumulating all store transfers
                    lo, hi = nts[0] * RB, (nts[-1] + 1) * RB
                    eng[(nts[0] // 4) % 2].dma_start(
                        out=o_d[img, cc * 128:(cc + 1) * 128, lo:hi, :],
                        in_=o_t[:, lo:hi, :],
                    )

            # warm the PE clock (p-state ramps over ~9us of activity) with
            # dummy matmuls on a memset tile while the first DMAs land
            # (~5.5us trigger-to-completion on a cold DMA engine)
            wu = wp.tile([128, NFREE], BF16)
            nc.gpsimd.memset(wu, 0)
            ps_warm = pp.tile([128, NFREE], F32, tag="ps", name="ps_warm")
            for _ in range(11):
                nc.tensor.matmul(ps_warm, wu[:, 0:128], wu,
                                 start=True, stop=True)

            for img in range(BPC):
                xs = load_img(img, first=(img == 0))
                for cc in range(2):
                    o_t = op.tile([128, H, W], BF16, tag="o",
                                  name=f"o_{img}_{cc}")
                    last = img == BPC - 1 and cc == 1
                    if last:
                        # taper the final passes so the drain tail is short
                        do_pass(xs, cc, o_t, img, [0, 1, 2, 3])
                        do_pass(xs, cc, o_t, img, [4, 5], fine=True)
                        do_pass(xs, cc, o_t, img, [6], fine=True)
                    else:
                        do_pass(xs, cc, o_t, img, [0, 1, 2, 3])
                        do_pass(xs, cc, o_t, img, [4, 5, 6])
    nc.compile()
    _CACHE["nc"] = nc
    return nc


def make_in_maps(inp, kernel, bias):
    xpad = np.zeros((B, CIN, HP, WP), np.float32)
    xpad[:, :, 1:1 + H, 1:1 + W] = inp
    xdev = xpad.astype(BF)
    # [cout, cin, kh, kw] -> [ci, cin_in, cc, tap*128+cout_in]
    kk = np.asarray(kernel, np.float32).reshape(2, 128, 2, 128, K, K)
    w_dev = np.ascontiguousarray(
        kk.transpose(2, 3, 0, 4, 5, 1).reshape(2, 128, 2, NW)
    ).astype(BF)
    b_dev = np.ascontiguousarray(np.asarray(bias, np.float32))
    return [
        {"x": np.ascontiguousarray(xdev[c * BPC:(c + 1) * BPC]),
         "w": w_dev, "b": b_dev}
        for c in range(NCORES)
    ]


def assemble(results):
    o = np.concatenate([np.asarray(results[c]["o"]) for c in range(NCORES)],
                       axis=0)
    return np.ascontiguousarray(o.astype(np.float32))


def kernel(inp, kernel, bias):
    nc = _build()
    in_maps = make_in_maps(inp, kernel, bias)
    r = run_bass_kernel_spmd(nc, in_maps, core_ids=list(range(NCORES)))
    return assemble(r.results)


# revision 22
# speedup vs baseline: 1.3521x; 1.0016x over previous
"""Trainium2 Bass kernel for nn_Conv2d (B=32, 256->256, 56x56, 3x3, pad=1) + bias.

Strategy
--------
Data-parallel over batch: 4 images per NeuronCore x 8 cores; weights/bias
replicated; no collectives.

Per core, the conv is computed as shifted matmuls on a host-padded input
(59 rows x 58 cols per image-channel): the tap (kh, kw) contribution to
output row-block [8n, 8n+8) is a matmul whose moving operand is the 2D
window x[:, 8n+kh : 8n+kh+8, kw : kw+56] (row stride 58) and whose PSUM
output is [8, 56] contiguous — so no junk columns are ever computed.
Operands are bf16 (rel err ~4e-3 vs the 2e-2 gate): bf16 self-loading
matmuls hide their LDWEIGHTS under the previous matmul's stream, unlike
f32r.  Each (img, cout-chunk) accumulates 18 (cin-chunk, tap) matmuls per
PSUM bank; banks are grouped 4+3 and ping-pong across the 8 PSUM banks so
evictions (bias-add on DVE, bf16 out) and stores overlap the next group's
matmuls.  Dummy matmuls on a memset tile warm the PE clock (p-state) while
the first DMAs land (~5us fixed DMA-engine startup latency).
"""

import numpy as np
import ml_dtypes

import concourse.bacc as bacc
import concourse.tile as tile
import concourse.mybir as mybir
from concourse.bass_utils import run_bass_kernel_spmd

F32 = mybir.dt.float32
BF16 = mybir.dt.bfloat16
BF = ml_dtypes.bfloat16

B, CIN, COUT, H, W, K = 32, 256, 256, 56, 56, 3
NCORES = 8
BPC = B // NCORES          # images per core
WP = W + 2                 # padded row width (58)
HP = H + 3                 # padded rows (59): 1 top, 2 bottom (tail tap reads)
NT = 7                     # output row-blocks per (img, cout-chunk)
RB = H // NT               # 8 output rows per block
NFREE = RB * W             # 448 output positions per matmul
NW = K * K * 128           # weight free length per (ci, cc): 9 taps x 128 couts

_CACHE = {}


def _build():
    if "nc" in _CACHE:
        return _CACHE["nc"]
    nc = bacc.Bacc("TRN2", target_bir_lowering=False, debug=False,
                   num_swdge_queues=1)
    x_d = nc.dram_tensor("x", [BPC, CIN, HP, WP], BF16,
                         kind="ExternalInput").ap()
    w_d = nc.dram_tensor("w", [2, 128, 2, NW], BF16, kind="ExternalInput").ap()
    b_d = nc.dram_tensor("b", [COUT], F32, kind="ExternalInput").ap()
    o_d = nc.dram_tensor("o", [BPC, COUT, H, W], BF16,
                         kind="ExternalOutput").ap()

    with tile.TileContext(nc) as tc:
        with (
            tc.tile_pool(name="wp", bufs=1) as wp,
            tc.tile_pool(name="xp", bufs=6) as xp,
            tc.tile_pool(name="op", bufs=2) as op,
            tc.tile_pool(name="pp", bufs=8, space="PSUM") as pp,
        ):
            # DMA trigger instructions cost ~0.65us EACH on the issuing
            # engine, so issue in parallel from both HWDGE engines:
            # sync carries ci=0 traffic, scalar carries ci=1.
            eng = [nc.sync, nc.scalar]

            bias_t = wp.tile([128, 2], F32)
            # weights [cin-in-chunk, ci, cc, tap*128+cout]: one contiguous
            # 295KB DMA per (ci, cc) chunk, split 3+6 taps for cc=0.
            w_t = wp.tile([128, 2, 2, NW], BF16)

            def w_dma(e, ci, cc, lo=0, hi=K * K):
                e.dma_start(out=w_t[:, ci, cc, lo * 128:hi * 128],
                            in_=w_d[ci, :, cc, lo * 128:hi * 128])

            def x_dma(e, xs, img, ci, lo, hi):
                e.dma_start(
                    out=xs[ci][:, lo:hi, :],
                    in_=x_d[img, ci * 128:(ci + 1) * 128, lo:hi, :],
                )

            # steady images: 4 coarse row-slices (they prefetch a whole
            # image ahead anyway)
            xsl = [0, 15, 30, 45, HP]

            def load_img0():
                # Hand-scheduled by need-by time: mi walks ci0 taps 0-8
                # (~0.75us each) then ci1, so ci=1 data has ~7us of slack —
                # the scalar engine fronts ci0's mid rows while sync lands
                # the critical rows [0:8)+w taps that gate the first matmul.
                xs = [xp.tile([128, HP, WP], BF16, tag="x", name=f"x_0_{ci}")
                      for ci in range(2)]
                s0, s1 = eng

                def X(e, ci, a, b):
                    x_dma(e, xs, 0, ci, a, b)

                # the first matmul wave (4 row-blocks) touches ci0 rows
                # 0-33 within ~1us, so scalar fronts rows 18-35 while sync
                # lands rows 0-18 + the gating weight taps
                X(s0, 0, 0, 8); w_dma(s0, 0, 0, 0, 1)
                X(s0, 0, 18, 27); w_dma(s0, 0, 0, 1, 3)
                w_dma(s0, 0, 0, 3, K * K)
                X(s0, 0, 35, 44); X(s0, 0, 44, HP)
                s0.dma_start(out=bias_t[:, 0:1], in_=b_d[0:128])
                w_dma(s0, 0, 1)

                X(s1, 0, 8, 18); X(s1, 0, 27, 35)
                X(s1, 1, 0, 8); w_dma(s1, 1, 0, 0, 3)
                X(s1, 1, 8, 18); X(s1, 1, 18, 27)
                w_dma(s1, 1, 0, 3, K * K)
                X(s1, 1, 27, 44); X(s1, 1, 44, HP)
                s1.dma_start(out=bias_t[:, 1:2], in_=b_d[128:256])
                w_dma(s1, 1, 1)
                return xs

            def load_img(img, first=False):
                if first:
                    return load_img0()
                xs = [xp.tile([128, HP, WP], BF16, tag="x",
                              name=f"x_{img}_{ci}") for ci in range(2)]
                for ci in range(2):
                    e = eng[ci]
                    for s in range(len(xsl) - 1):
                        x_dma(e, xs, img, ci, xsl[s], xsl[s + 1])
                return xs

            def do_pass(xs, cc, o_t, img, nts, fine=False):
                """One PSUM accumulation wave over row-blocks `nts`: 18
                (ci, tap) x len(nts) matmuls, weight-outermost so the bf16
                self-load LDWEIGHTS amortizes over len(nts) streams."""
                pss = [pp.tile([128, RB, W], F32, tag="ps",
                               name=f"ps_{img}_{cc}_{nt}") for nt in nts]
                for mi, (ci, t) in enumerate(
                    [(ci, t) for ci in range(2) for t in range(K * K)]
                ):
                    kh, kw = divmod(t, K)
                    wsl = w_t[:, ci, cc, t * 128:(t + 1) * 128]
                    for ps, nt in zip(pss, nts):
                        r0 = nt * RB + kh
                        nc.tensor.matmul(
                            ps, wsl, xs[ci][:, r0:r0 + RB, kw:kw + W],
                            start=(mi == 0), stop=(mi == 17),
                        )
                # bias-add + PSUM eviction on the otherwise-idle DVE,
                # bf16 out halves store traffic
                for j, (ps, nt) in enumerate(zip(pss, nts)):
                    if fine and len(nts) == 1:
                        # final bank: half-row-block evicts + stores on both
                        # engines to shorten the drain tail
                        hb = RB // 2
                        for h in range(2):
                            r = nt * RB + h * hb
                            if h == 0:
                                nc.vector.tensor_scalar_add(
                                    o_t[:, r:r + hb, :],
                                    ps[:, h * hb:(h + 1) * hb, :],
                                    bias_t[:, cc:cc + 1],
                                )
                            else:
                                # second half on ScalarE so both evicts run
                                # concurrently in the drain tail
                                nc.scalar.activation(
                                    o_t[:, r:r + hb, :],
                                    ps[:, h * hb:(h + 1) * hb, :],
                                    mybir.ActivationFunctionType.Identity,
                                    bias=bias_t[:, cc:cc + 1],
                                )
                            eng[h].dma_start(
                                out=o_d[img, cc * 128:(cc + 1) * 128,
                                        r:r + hb, :],
                                in_=o_t[:, r:r + hb, :],
                            )
                        continue
                    nc.vector.tensor_scalar_add(
                        o_t[:, nt * RB:(nt + 1) * RB, :],
                        ps,
                        bias_t[:, cc:cc + 1],
                    )
                    if fine:
                        eng[j % 2].dma_start(
                            out=o_d[img, cc * 128:(cc + 1) * 128,
                                    nt * RB:(nt + 1) * RB, :],
                            in_=o_t[:, nt * RB:(nt + 1) * RB, :],
                        )
                if not fine:
                    # region A -> sync, region B -> scalar: keeps either
                    # HWDGE queue from accumulating all store transfers
                    lo, hi = nts[0] * RB, (nts[-1] + 1) * RB
                    eng[(nts[0] // 4) % 2].dma_start(
                        out=o_d[img, cc * 128:(cc + 1) * 128, lo:hi, :],
                        in_=o_t[:, lo:hi, :],
                    )

            # warm the PE clock (p-state ramps over ~9us of activity) with
            # dummy matmuls on a memset tile while the first DMAs land
            # (~5.5us trigger-to-completion on a cold DMA engine)
            wu = wp.tile([128, NFREE], BF16)
            nc.gpsimd.memset(wu, 0)
            ps_warm = pp.tile([128, NFREE], F32, tag="ps", name="ps_warm")
            for _ in range(11):
                nc.tensor.matmul(ps_warm, wu[:, 0:128], wu,
                                 start=True, stop=True)

            for img in range(BPC):
                xs = load_img(img, first=(img == 0))
                for cc in range(2):
                    o_t = op.tile([128, H, W], BF16, tag="o",
                                  name=f"o_{img}_{cc}")
                    last = img == BPC - 1 and cc == 1
                    if last:
                        # taper the final passes so the drain tail is short
                        do_pass(xs, cc, o_t, img, [0, 1, 2, 3])
                        do_pass(xs, cc, o_t, img, [4, 5], fine=True)
                        do_pass(xs, cc, o_t, img, [6], fine=True)
                    else:
                        do_pass(xs, cc, o_t, img, [0, 1, 2, 3])
                        do_pass(xs, cc, o_t, img, [4, 5, 6])
    nc.compile()
    _CACHE["nc"] = nc
    return nc


def make_in_maps(inp, kernel, bias):
    xpad = np.zeros((B, CIN, HP, WP), np.float32)
    xpad[:, :, 1:1 + H, 1:1 + W] = inp
    xdev = xpad.astype(BF)
    # [cout, cin, kh, kw] -> [ci, cin_in, cc, tap*128+cout_in]
    kk = np.asarray(kernel, np.float32).reshape(2, 128, 2, 128, K, K)
    w_dev = np.ascontiguousarray(
        kk.transpose(2, 3, 0, 4, 5, 1).reshape(2, 128, 2, NW)
    ).astype(BF)
    b_dev = np.ascontiguousarray(np.asarray(bias, np.float32))
    return [
        {"x": np.ascontiguousarray(xdev[c * BPC:(c + 1) * BPC]),
         "w": w_dev, "b": b_dev}
        for c in range(NCORES)
    ]


def assemble(results):
    o = np.concatenate([np.asarray(results[c]["o"]) for c in range(NCORES)],
                       axis=0)
    return np.ascontiguousarray(o.astype(np.float32))


def kernel(inp, kernel, bias):
    nc = _build()
    in_maps = make_in_maps(inp, kernel, bias)
    r = run_bass_kernel_spmd(nc, in_maps, core_ids=list(range(NCORES)))
    return assemble(r.results)


# revision 23
# speedup vs baseline: 1.3577x; 1.0041x over previous
"""Trainium2 Bass kernel for nn_Conv2d (B=32, 256->256, 56x56, 3x3, pad=1) + bias.

Strategy
--------
Data-parallel over batch: 4 images per NeuronCore x 8 cores; weights/bias
replicated; no collectives.

Per core, the conv is computed as shifted matmuls on a host-padded input
(59 rows x 58 cols per image-channel): the tap (kh, kw) contribution to
output row-block [8n, 8n+8) is a matmul whose moving operand is the 2D
window x[:, 8n+kh : 8n+kh+8, kw : kw+56] (row stride 58) and whose PSUM
output is [8, 56] contiguous — so no junk columns are ever computed.
Operands are bf16 (rel err ~4e-3 vs the 2e-2 gate): bf16 self-loading
matmuls hide their LDWEIGHTS under the previous matmul's stream, unlike
f32r.  Each (img, cout-chunk) accumulates 18 (cin-chunk, tap) matmuls per
PSUM bank; banks are grouped 4+3 and ping-pong across the 8 PSUM banks so
evictions (bias-add on DVE, bf16 out) and stores overlap the next group's
matmuls.  Dummy matmuls on a memset tile warm the PE clock (p-state) while
the first DMAs land (~5us fixed DMA-engine startup latency).
"""

import numpy as np
import ml_dtypes

import concourse.bacc as bacc
import concourse.tile as tile
import concourse.mybir as mybir
from concourse.bass_utils import run_bass_kernel_spmd

F32 = mybir.dt.float32
BF16 = mybir.dt.bfloat16
BF = ml_dtypes.bfloat16

B, CIN, COUT, H, W, K = 32, 256, 256, 56, 56, 3
NCORES = 8
BPC = B // NCORES          # images per core
WP = W + 2                 # padded row width (58)
HP = H + 3                 # padded rows (59): 1 top, 2 bottom (tail tap reads)
NT = 7                     # output row-blocks per (img, cout-chunk)
RB = H // NT               # 8 output rows per block
NFREE = RB * W             # 448 output positions per matmul
NW = K * K * 128           # weight free length per (ci, cc): 9 taps x 128 couts

_CACHE = {}


def _build():
    if "nc" in _CACHE:
        return _CACHE["nc"]
    nc = bacc.Bacc("TRN2", target_bir_lowering=False, debug=False,
                   num_swdge_queues=1)
    x_d = nc.dram_tensor("x", [BPC, CIN, HP, WP], BF16,
                         kind="ExternalInput").ap()
    w_d = nc.dram_tensor("w", [2, 128, 2, NW], BF16, kind="ExternalInput").ap()
    b_d = nc.dram_tensor("b", [COUT], F32, kind="ExternalInput").ap()
    o_d = nc.dram_tensor("o", [BPC, COUT, H, W], BF16,
                         kind="ExternalOutput").ap()

    with tile.TileContext(nc) as tc:
        with (
            tc.tile_pool(name="wp", bufs=1) as wp,
            tc.tile_pool(name="xp", bufs=4) as xp,
            tc.tile_pool(name="op", bufs=2) as op,
            tc.tile_pool(name="pp", bufs=8, space="PSUM") as pp,
        ):
            # DMA trigger instructions cost ~0.65us EACH on the issuing
            # engine, so issue in parallel from both HWDGE engines:
            # sync carries ci=0 traffic, scalar carries ci=1.
            eng = [nc.sync, nc.scalar]

            bias_t = wp.tile([128, 2], F32)
            # weights [cin-in-chunk, ci, cc, tap*128+cout]: one contiguous
            # 295KB DMA per (ci, cc) chunk, split 3+6 taps for cc=0.
            w_t = wp.tile([128, 2, 2, NW], BF16)

            def w_dma(e, ci, cc, lo=0, hi=K * K):
                e.dma_start(out=w_t[:, ci, cc, lo * 128:hi * 128],
                            in_=w_d[ci, :, cc, lo * 128:hi * 128])

            def x_dma(e, xs, img, ci, lo, hi):
                e.dma_start(
                    out=xs[ci][:, lo:hi, :],
                    in_=x_d[img, ci * 128:(ci + 1) * 128, lo:hi, :],
                )

            # steady images: 4 coarse row-slices (they prefetch a whole
            # image ahead anyway)
            xsl = [0, 15, 30, 45, HP]

            def load_img0():
                # Hand-scheduled by need-by time: mi walks ci0 taps 0-8
                # (~0.75us each) then ci1, so ci=1 data has ~7us of slack —
                # the scalar engine fronts ci0's mid rows while sync lands
                # the critical rows [0:8)+w taps that gate the first matmul.
                xs = [xp.tile([128, HP, WP], BF16, tag="x", name=f"x_0_{ci}")
                      for ci in range(2)]
                s0, s1 = eng

                def X(e, ci, a, b):
                    x_dma(e, xs, 0, ci, a, b)

                # the first matmul wave (4 row-blocks) touches ci0 rows
                # 0-33 within ~1us, so scalar fronts rows 18-35 while sync
                # lands rows 0-18 + the gating weight taps
                X(s0, 0, 0, 8); w_dma(s0, 0, 0, 0, 1)
                X(s0, 0, 18, 27); w_dma(s0, 0, 0, 1, 3)
                w_dma(s0, 0, 0, 3, K * K)
                X(s0, 0, 35, 44); X(s0, 0, 44, HP)
                s0.dma_start(out=bias_t[:, 0:1], in_=b_d[0:128])
                w_dma(s0, 0, 1)

                X(s1, 0, 8, 18); X(s1, 0, 27, 35)
                X(s1, 1, 0, 8); w_dma(s1, 1, 0, 0, 3)
                X(s1, 1, 8, 18); X(s1, 1, 18, 27)
                w_dma(s1, 1, 0, 3, K * K)
                X(s1, 1, 27, 44); X(s1, 1, 44, HP)
                s1.dma_start(out=bias_t[:, 1:2], in_=b_d[128:256])
                w_dma(s1, 1, 1)
                return xs

            def load_img(img, first=False):
                if first:
                    return load_img0()
                xs = [xp.tile([128, HP, WP], BF16, tag="x",
                              name=f"x_{img}_{ci}") for ci in range(2)]
                for ci in range(2):
                    e = eng[ci]
                    for s in range(len(xsl) - 1):
                        x_dma(e, xs, img, ci, xsl[s], xsl[s + 1])
                return xs

            def do_pass(xs, cc, o_t, img, nts, fine=False):
                """One PSUM accumulation wave over row-blocks `nts`: 18
                (ci, tap) x len(nts) matmuls, weight-outermost so the bf16
                self-load LDWEIGHTS amortizes over len(nts) streams."""
                pss = [pp.tile([128, RB, W], F32, tag="ps",
                               name=f"ps_{img}_{cc}_{nt}") for nt in nts]
                for mi, (ci, t) in enumerate(
                    [(ci, t) for ci in range(2) for t in range(K * K)]
                ):
                    kh, kw = divmod(t, K)
                    wsl = w_t[:, ci, cc, t * 128:(t + 1) * 128]
                    for ps, nt in zip(pss, nts):
                        r0 = nt * RB + kh
                        nc.tensor.matmul(
                            ps, wsl, xs[ci][:, r0:r0 + RB, kw:kw + W],
                            start=(mi == 0), stop=(mi == 17),
                        )
                # bias-add + PSUM eviction on the otherwise-idle DVE,
                # bf16 out halves store traffic
                for j, (ps, nt) in enumerate(zip(pss, nts)):
                    if fine and len(nts) == 1:
                        # final bank: half-row-block evicts + stores on both
                        # engines to shorten the drain tail
                        hb = RB // 2
                        for h in range(2):
                            r = nt * RB + h * hb
                            if h == 0:
                                nc.vector.tensor_scalar_add(
                                    o_t[:, r:r + hb, :],
                                    ps[:, h * hb:(h + 1) * hb, :],
                                    bias_t[:, cc:cc + 1],
                                )
                            else:
                                # second half on ScalarE so both evicts run
                                # concurrently in the drain tail
                                nc.scalar.activation(
                                    o_t[:, r:r + hb, :],
                                    ps[:, h * hb:(h + 1) * hb, :],
                                    mybir.ActivationFunctionType.Identity,
                                    bias=bias_t[:, cc:cc + 1],
                                )
                            eng[h].dma_start(
                                out=o_d[img, cc * 128:(cc + 1) * 128,
                                        r:r + hb, :],
                                in_=o_t[:, r:r + hb, :],
                            )
                        continue
                    nc.vector.tensor_scalar_add(
                        o_t[:, nt * RB:(nt + 1) * RB, :],
                        ps,
                        bias_t[:, cc:cc + 1],
                    )
                    if fine:
                        eng[j % 2].dma_start(
                            out=o_d[img, cc * 128:(cc + 1) * 128,
                                    nt * RB:(nt + 1) * RB, :],
                            in_=o_t[:, nt * RB:(nt + 1) * RB, :],
                        )
                if not fine:
                    # region A -> sync, region B -> scalar: keeps either
                    # HWDGE queue from accumulating all store transfers
                    lo, hi = nts[0] * RB, (nts[-1] + 1) * RB
                    eng[(nts[0] // 4) % 2].dma_start(
                        out=o_d[img, cc * 128:(cc + 1) * 128, lo:hi, :],
                        in_=o_t[:, lo:hi, :],
                    )

            # warm the PE clock (p-state ramps over ~9us of activity) with
            # dummy matmuls on a memset tile while the first DMAs land
            # (~5.5us trigger-to-completion on a cold DMA engine)
            wu = wp.tile([128, NFREE], BF16)
            nc.gpsimd.memset(wu, 0)
            ps_warm = pp.tile([128, NFREE], F32, tag="ps", name="ps_warm")
            for _ in range(11):
                nc.tensor.matmul(ps_warm, wu[:, 0:128], wu,
                                 start=True, stop=True)

            for img in range(BPC):
                xs = load_img(img, first=(img == 0))
                for cc in range(2):
                    o_t = op.tile([128, H, W], BF16, tag="o",
                                  name=f"o_{img}_{cc}")
                    last = img == BPC - 1 and cc == 1
                    if last:
                        # taper the final passes so the drain tail is short
                        do_pass(xs, cc, o_t, img, [0, 1, 2, 3])
                        do_pass(xs, cc, o_t, img, [4, 5], fine=True)
                        do_pass(xs, cc, o_t, img, [6], fine=True)
                    else:
                        do_pass(xs, cc, o_t, img, [0, 1, 2, 3])
                        do_pass(xs, cc, o_t, img, [4, 5, 6])
    nc.compile()
    _CACHE["nc"] = nc
    return nc


def make_in_maps(inp, kernel, bias):
    xpad = np.zeros((B, CIN, HP, WP), np.float32)
    xpad[:, :, 1:1 + H, 1:1 + W] = inp
    xdev = xpad.astype(BF)
    # [cout, cin, kh, kw] -> [ci, cin_in, cc, tap*128+cout_in]
    kk = np.asarray(kernel, np.float32).reshape(2, 128, 2, 128, K, K)
    w_dev = np.ascontiguousarray(
        kk.transpose(2, 3, 0, 4, 5, 1).reshape(2, 128, 2, NW)
    ).astype(BF)
    b_dev = np.ascontiguousarray(np.asarray(bias, np.float32))
    return [
        {"x": np.ascontiguousarray(xdev[c * BPC:(c + 1) * BPC]),
         "w": w_dev, "b": b_dev}
        for c in range(NCORES)
    ]


def assemble(results):
    o = np.concatenate([np.asarray(results[c]["o"]) for c in range(NCORES)],
                       axis=0)
    return np.ascontiguousarray(o.astype(np.float32))


def kernel(inp, kernel, bias):
    nc = _build()
    in_maps = make_in_maps(inp, kernel, bias)
    r = run_bass_kernel_spmd(nc, in_maps, core_ids=list(range(NCORES)))
    return assemble(r.results)
